# revision 1
# baseline (speedup 1.0000x reference)
"""Trainium2 Bass kernel for nn_EncoderLayer_88476326298146 (sparse graph attention).

Row-sharded across 8 NeuronCores: core c owns nodes [c*2048, (c+1)*2048) and the
edges targeting them (host-sorted by row into 16 windows of 128 rows, padded to a
fixed per-window count TW). k/v (bf16, [-k|v] rows, k negated) are replicated via
AllGather; per-edge col features come from dma_gather.

Engine-balanced v2:
- LN affine (g,b) folded into the following weight matrices on host; LN stats via
  bn_stats/bn_aggr; biases applied via rank-1 ones-row matmuls on the PE.
- diff = q_row - k_col computed on the PE (s2^T@q then accumulate identity@(-k));
  square on the Act engine from PSUM; per-head reduce on DVE.
- exp emitted as bf16 PAIRS so the alpha*v multiply runs in DVE 2x mode.
- segment softmax with m=0 (scores <= max pos_att_bias, exp bounded); segment sums
  via host-built one-hot matrices on the PE.
- FFN1 produced transposed (lhsT=W1 tiles, rhs=z2^T) with gelu+bias fused on Act;
  FFN2 consumes h^T directly as lhsT: zero transposes in the FFN.
- All intermediates (x, x2, z^T, q, h^T) stay in SBUF; only k/v round-trips DRAM
  for the AllGather + gather.
"""
import os
import numpy as np

import concourse.bass as bass
import concourse.bacc as bacc
import concourse.mybir as mybir
import concourse.tile as tile
from concourse.bass_utils import run_bass_kernel_spmd
from concourse.library_config import mlp as mlp_lib

L, E, SP, C, H, DH, HID = 16384, 131072, 20000, 512, 8, 64, 1024
NCORES = 8
RS = L // NCORES
NW = RS // 128
P = 128
F32 = mybir.dt.float32
BF16 = mybir.dt.bfloat16
I16 = mybir.dt.int16
FP8 = mybir.dt.float8e4
WS = 64.0  # weight pre-scale (fp8 subnormal avoidance); descaled in Act casts
AF = mybir.ActivationFunctionType
ALU = mybir.AluOpType
AX = mybir.AxisListType

_cache = {}
_SKIP = set(os.environ.get("KSKIP", "").split(","))


def _build(TW):
    NT = TW // P
    GS = 3  # tiles per score group (PSUM: GS banks for qe)
    inv_s = 1.0 / float(np.sqrt(DH))
    nc = bacc.Bacc("TRN2", target_bir_lowering=False, debug=False, num_devices=NCORES)

    x_in = nc.dram_tensor("x_in", [RS, C], F32, kind="ExternalInput")
    w_qkv = nc.dram_tensor("w_qkv", [C, 3 * C], FP8, kind="ExternalInput")
    w_o = nc.dram_tensor("w_o", [C, C], FP8, kind="ExternalInput")
    w_1 = nc.dram_tensor("w_1", [C, HID], FP8, kind="ExternalInput")
    w_2 = nc.dram_tensor("w_2", [HID, C], FP8, kind="ExternalInput")
    w_vec4 = nc.dram_tensor("w_vec4", [32, C], BF16, kind="ExternalInput")
    b1_col = nc.dram_tensor("b1_col", [P, HID // P], F32, kind="ExternalInput")
    brows = nc.dram_tensor("brows", [1, 4 * C], FP8, kind="ExternalInput")
    ident_in = nc.dram_tensor("ident_in", [P, P], BF16, kind="ExternalInput")
    ones_in = nc.dram_tensor("ones_in", [1, P], FP8, kind="ExternalInput")
    eps_in = nc.dram_tensor("eps_in", [P, 1], F32, kind="ExternalInput")
    eidx = nc.dram_tensor("eidx", [NW, P, TW // 16], I16, kind="ExternalInput")
    rel4 = nc.dram_tensor("rel4", [NW, P, NT, 4], BF16, kind="ExternalInput")
    biasC = nc.dram_tensor("biasC", [NW, P, NT, 8], F32, kind="ExternalInput")
    s_e2r = nc.dram_tensor("s_e2r", [NW, P, NT, P], BF16, kind="ExternalInput")
    s_r2e = nc.dram_tensor("s_r2e", [NW, P, NT, P], BF16, kind="ExternalInput")
    y_out = nc.dram_tensor("y_out", [RS, C], F32, kind="ExternalOutput")
    dbg_out = nc.dram_tensor("dbg_out", [P, NW, HID], BF16, kind="ExternalOutput")
    dbg2_out = nc.dram_tensor("dbg2_out", [P, 2048], F32, kind="ExternalOutput")

    x_t = x_in.ap().rearrange("(m p) n -> p m n", p=P)
    y_t = y_out.ap().rearrange("(m p) n -> p m n", p=P)

    with tile.TileContext(nc) as tc:
        with tc.tile_pool(name="dram", bufs=1, space="DRAM") as dram, \
             tc.tile_pool(name="const", bufs=1) as const:
            nc.gpsimd.load_library(mlp_lib)

            ident = const.tile([P, P], BF16)
            nc.sync.dma_start(ident[:], ident_in.ap())
            ones_s = const.tile([1, P], FP8)
            nc.sync.dma_start(ones_s[:], ones_in.ap())
            eps_t = const.tile([P, 1], F32)
            nc.sync.dma_start(eps_t[:], eps_in.ap())
            brow_s = const.tile([1, 4 * C], FP8)
            nc.sync.dma_start(brow_s[:], brows.ap())
            wvec_s = const.tile([32, C], BF16)
            nc.sync.dma_start(wvec_s[:], w_vec4.ap())
            b1c_s = const.tile([P, HID // P], F32)
            nc.sync.dma_start(b1c_s[:], b1_col.ap())

            x_all = const.tile([P, NW, C], F32)
            x2_all = const.tile([P, NW, C], BF16)
            zt_sbuf = const.tile([P, C // P, RS], FP8)
            q_sbuf = const.tile([P, NW, C], BF16)
            hT_sbuf = const.tile([P, HID // P, RS], FP8)

            kv_shard = dram.tile([RS, 2 * C], BF16)
            if "ag" not in _SKIP:
                kvt = dram.tile([L, 2 * C], BF16, addr_space="Shared")
            else:
                kvt = dram.tile([L, 2 * C], BF16)

            # ---------- LN helper: stats + normalized bf16 z (no affine) ----
            def ln_win(wk, ptp, src, m, copy_eng="v"):
                stats = wk.tile([P, 6], F32, tag="ln_st")
                nc.vector.bn_stats(stats[:], src)
                mv = wk.tile([P, 2], F32, tag="ln_mv")
                nc.vector.bn_aggr(mv[:], stats[:])
                sd = wk.tile([P, 1], F32, tag="ln_sd")
                nc.scalar.activation(sd[:], mv[:, 1:2], AF.Sqrt, bias=eps_t[:], scale=1.0)
                rs_ = wk.tile([P, 1], F32, tag="ln_rs")
                nc.vector.reciprocal(rs_[:], sd[:])
                nmr = wk.tile([P, 1], F32, tag="ln_nmr")
                nc.vector.scalar_tensor_tensor(
                    out=nmr[:], in0=mv[:, 0:1], scalar=-1.0, in1=rs_[:],
                    op0=ALU.mult, op1=ALU.mult)
                zf = wk.tile([P, C], BF16, tag="ln_zf")
                nc.scalar.activation(zf[:], src, AF.Identity, bias=nmr[:], scale=rs_[:])
                tp = ptp.tile([P, C], BF16, tag="tp")
                for c4 in range(C // P):
                    nc.tensor.transpose(tp[:, c4 * P:(c4 + 1) * P],
                                        zf[:, c4 * P:(c4 + 1) * P], ident[:])
                if copy_eng == "a":
                    nc.scalar.activation(
                        zt_sbuf[:, :, m * P:(m + 1) * P],
                        tp[:].rearrange("p (c4 q) -> p c4 q", q=P), AF.Identity)
                else:
                    nc.vector.tensor_copy(
                        zt_sbuf[:, :, m * P:(m + 1) * P],
                        tp[:].rearrange("p (c4 q) -> p c4 q", q=P))

            # ============ P1: LN1 + QKV ============
            if "p1" not in _SKIP:
             with tc.tile_pool(name="p1", bufs=2) as wk, \
                 tc.tile_pool(name="p1c", bufs=1) as cst, \
                 tc.tile_pool(name="p1tp", bufs=2, space="PSUM") as ptp, \
                 tc.tile_pool(name="p1ps", bufs=2, space="PSUM") as pps:
                wqkv_s = cst.tile([P, C // P, 3 * C], FP8, name="wqkv")
                nc.gpsimd.dma_start(wqkv_s[:], w_qkv.ap().rearrange("(ko p) n -> p ko n", p=P))
                kv_sh_t = kv_shard[:].rearrange("(m p) n -> p m n", p=P)
                for m in range(NW):
                    nc.sync.dma_start(x_all[:, m, :], x_t[:, m, :])
                    ln_win(wk, ptp, x_all[:, m, :], m, copy_eng="a")
                    for nb in range(3):
                        ps = pps.tile([P, C], F32, tag="ps")
                        for ko in range(0, C // P, 2):
                            nc.tensor.matmul(
                                ps[:],
                                lhsT=zt_sbuf[:, ko:ko + 2, m * P:(m + 1) * P],
                                rhs=wqkv_s[:, ko:ko + 2, nb * C:(nb + 1) * C],
                                start=(ko == 0), stop=(ko == C // P - 2 and nb != 0),
                                perf_mode=mybir.MatmulPerfMode.DoubleRow)
                        if nb == 0:
                            nc.tensor.matmul(ps[:], lhsT=ones_s[:], rhs=brow_s[0:1, 0:C],
                                             start=False, stop=True)
                            nc.scalar.activation(q_sbuf[:, m, :], ps[:], AF.Identity,
                                                 scale=1.0 / WS)
                        else:
                            kvb = wk.tile([P, C], BF16, tag="kvb")
                            # k stored negated so the edge phase can accumulate
                            # qe + (-k) on the PE via an identity matmul.
                            nc.vector.tensor_scalar_mul(
                                kvb[:], ps[:], (-1.0 if nb == 1 else 1.0) / WS)
                            nc.sync.dma_start(
                                kv_sh_t[:, m, (nb - 1) * C:nb * C], kvb[:])

            # ============ P2: AllGather ============
            if "ag" not in _SKIP:
                nc.gpsimd.collective_compute(
                    "AllGather", ALU.bypass, replica_groups=[list(range(NCORES))],
                    ins=[kv_shard[:].opt()], outs=[kvt[:].opt()])

            # ============ P3: edge windows + Wo + residual ============
            if "edge" not in _SKIP:
             with tc.tile_pool(name="big", bufs=2) as big, \
                 tc.tile_pool(name="ew", bufs=2) as wk, \
                 tc.tile_pool(name="pqe", bufs=1, space="PSUM") as pqe, \
                 tc.tile_pool(name="ppsW", bufs=1, space="PSUM") as ppsW, \
                 tc.tile_pool(name="ptpc", bufs=1, space="PSUM") as ptpc, \
                 tc.tile_pool(name="p5ps", bufs=2, space="PSUM") as p5ps, \
                 tc.tile_pool(name="woc", bufs=1) as woc:
                wo_s = woc.tile([P, C // P, C], FP8, name="wo")
                nc.gpsimd.dma_start(wo_s[:], w_o.ap().rearrange("(ko p) n -> p ko n", p=P))
                for w in range(NW):
                    idx_t = big.tile([P, TW // 16], I16, tag="idx", bufs=3)
                    nc.gpsimd.dma_start(idx_t[:], eidx.ap()[w])
                    kv_g = big.tile([P, NT, 2 * C], BF16, tag="kv", bufs=3)
                    nc.gpsimd.dma_gather(
                        out_ap=kv_g[:], in_ap=kvt[:], idxs_ap=idx_t[:],
                        num_idxs=TW, num_idxs_reg=TW, elem_size=2 * C,
                        single_packet=False)
                    s1_t = big.tile([P, NT, P], BF16, tag="s1")
                    nc.sync.dma_start(s1_t[:], s_e2r.ap()[w])
                    s2_t = big.tile([P, NT, P], BF16, tag="s2")
                    nc.sync.dma_start(s2_t[:], s_r2e.ap()[w])
                    rel_t = big.tile([P, NT, 4], BF16, tag="rel")
                    nc.sync.dma_start(rel_t[:], rel4.ap()[w])
                    bias_t = big.tile([P, NT, 8], F32, tag="bias")
                    nc.sync.dma_start(bias_t[:], biasC.ap()[w])

                    psW = ppsW.tile([P, 560], F32, tag="psW")
                    for t0 in range(0, NT, GS):
                        tb = min(GS, NT - t0)
                        qe = pqe.tile([P, GS, C], F32, tag="qe")
                        for d_ in range(tb):
                            nc.tensor.matmul(qe[:, d_, :], lhsT=s2_t[:, t0 + d_, :],
                                             rhs=q_sbuf[:, w, :], start=True, stop=False)
                            nc.tensor.matmul(qe[:, d_, :], lhsT=ident[:],
                                             rhs=kv_g[:, t0 + d_, 0:C],
                                             start=False, stop=True)
                        dsq = wk.tile([P, GS, C], BF16, tag="dsq")
                        nc.scalar.activation(dsq[:, 0:tb, :], qe[:, 0:tb, :], AF.Square)
                        s8 = wk.tile([P, GS, H], F32, tag="s8")
                        nc.vector.reduce_sum(
                            s8[:, 0:tb, :],
                            dsq[:, 0:tb, :].rearrange("p t (h d) -> p t h d", h=H),
                            axis=AX.X)
                        sc = wk.tile([P, GS, H], F32, tag="sc")
                        nc.vector.scalar_tensor_tensor(
                            out=sc[:, 0:tb, :], in0=s8[:, 0:tb, :], scalar=-inv_s,
                            in1=bias_t[:, t0:t0 + tb, :], op0=ALU.mult, op1=ALU.add)
                        eaux = wk.tile([P, GS, 48], BF16, tag="eaux")
                        nc.scalar.activation(
                            eaux[:, 0:tb, 0:16].rearrange("p t (h j) -> p t h j", h=H),
                            sc[:, 0:tb, :].unsqueeze(3).broadcast_to([P, tb, H, 2]),
                            AF.Exp)
                        exp2 = wk.tile([P, GS, 16], BF16, tag="exp2")
                        nc.scalar.activation(
                            exp2[:, 0:tb, :].rearrange("p t (h j) -> p t h j", h=H),
                            sc[:, 0:tb, :].unsqueeze(3).broadcast_to([P, tb, H, 2]),
                            AF.Exp)
                        if w == 0 and t0 == 0 and os.environ.get("KDBG") == "edge1":
                            d2 = wk.tile([P, 2048], F32, tag="d2")
                            nc.vector.tensor_copy(d2[:, 0:24], s8[:].rearrange("p t h -> p (t h)"))
                            nc.vector.tensor_copy(d2[:, 24:48], sc[:].rearrange("p t h -> p (t h)"))
                            nc.vector.tensor_copy(d2[:, 48:560], qe[:, 0, :])
                            nc.vector.tensor_copy(d2[:, 560:584], bias_t[:, 0:3, :].rearrange("p t h -> p (t h)"))
                            nc.sync.dma_start(dbg2_out.ap(), d2[:])
                        pev = wk.tile([P, GS, C], BF16, tag="pev")
                        for d_ in range(tb):
                            nc.vector.tensor_mul(
                                pev[:, d_, :].rearrange("p (h a j) -> p h a j", h=H, j=2),
                                exp2[:, d_, :].rearrange("p (h j) -> p h j", h=H)
                                    .unsqueeze(2).broadcast_to([P, H, DH // 2, 2]),
                                kv_g[:, t0 + d_, C:2 * C]
                                    .rearrange("p (h a j) -> p h a j", h=H, j=2))
                        nc.vector.tensor_mul(
                            eaux[:, 0:tb, 16:48].rearrange("p t (h a) -> p t h a", h=H),
                            eaux[:, 0:tb, 0:16].rearrange("p t (h j) -> p t h j", h=H)[:, :, :, 0:1]
                                .broadcast_to([P, tb, H, 4]),
                            rel_t[:, t0:t0 + tb, :].unsqueeze(2)
                                .broadcast_to([P, tb, H, 4]))
                        for d_ in range(tb):
                            t = t0 + d_
                            nc.tensor.matmul(psW[:, 0:512], lhsT=s1_t[:, t, :],
                                             rhs=pev[:, d_, :],
                                             start=(t == 0), stop=False)
                            nc.tensor.matmul(psW[:, 512:560], lhsT=s1_t[:, t, :],
                                             rhs=eaux[:, d_, :],
                                             start=(t == 0), stop=(t == NT - 1))

                    den = wk.tile([P, 16], F32, tag="den")
                    nc.vector.tensor_scalar_max(den[:], psW[:, 512:528], 1e-30)
                    rden = wk.tile([P, 16], F32, tag="rden")
                    nc.vector.reciprocal(rden[:], den[:])
                    # fold the (unnormalized) Wvec term into psW[0:512]: w_vec4 is
                    # head-block-diagonal, so the per-(row,head) rden factors
                    # through the sum.
                    anr = wk.tile([P, 32], BF16, tag="anr")
                    nc.scalar.activation(anr[:], psW[:, 528:560], AF.Identity)
                    tpc = ptpc.tile([P, C], BF16, tag="tpc")
                    nc.tensor.transpose(tpc[0:32, 0:P], anr[:], ident[:])
                    an_ts = wk.tile([32, P], BF16, tag="an_ts")
                    nc.scalar.activation(an_ts[:], tpc[0:32, 0:P], AF.Identity)
                    nc.tensor.matmul(psW[:, 0:512], lhsT=an_ts[:], rhs=wvec_s[:],
                                     start=False, stop=True)
                    attin = wk.tile([P, C], BF16, tag="attin")
                    nc.vector.tensor_mul(
                        attin[:].rearrange("p (h d) -> p h d", h=H),
                        psW[:, 0:512].rearrange("p (h d) -> p h d", h=H),
                        rden[:].rearrange("p (h j) -> p h j", h=H)[:, :, 0:1]
                            .broadcast_to([P, H, DH]))
                    tpa = ptpc.tile([P, C], BF16, tag="tpc")
                    for c4 in range(C // P):
                        nc.tensor.transpose(tpa[:, c4 * P:(c4 + 1) * P],
                                            attin[:, c4 * P:(c4 + 1) * P], ident[:])
                    at_sb = wk.tile([P, C // P, P], FP8, tag="at_sb")
                    nc.scalar.activation(
                        at_sb[:], tpa[:].rearrange("p (c4 q) -> p c4 q", q=P),
                        AF.Identity)
                    x2ps = p5ps.tile([P, C], F32, tag="p5")
                    for ko in range(0, C // P, 2):
                        nc.tensor.matmul(x2ps[:], lhsT=at_sb[:, ko:ko + 2, :],
                                         rhs=wo_s[:, ko:ko + 2, :],
                                         start=(ko == 0), stop=False,
                                         perf_mode=mybir.MatmulPerfMode.DoubleRow)
                    nc.tensor.matmul(x2ps[:], lhsT=ones_s[:], rhs=brow_s[0:1, C:2 * C],
                                     start=False, stop=True)
                    nc.vector.scalar_tensor_tensor(
                        out=x2_all[:, w, :], in0=x2ps[:], scalar=1.0 / WS,
                        in1=x_all[:, w, :], op0=ALU.mult, op1=ALU.add)

            # ============ P4: LN2 + FFN (fused, per row-chunk) ============
            if "p4" not in _SKIP:
             with tc.tile_pool(name="p4", bufs=2) as wk, \
                 tc.tile_pool(name="p4c", bufs=1) as cst, \
                 tc.tile_pool(name="p4tp", bufs=2, space="PSUM") as ptp, \
                 tc.tile_pool(name="f1ps", bufs=2, space="PSUM") as pps1, \
                 tc.tile_pool(name="f2ps", bufs=2, space="PSUM") as pps2:
                w1_s = cst.tile([P, C // P, HID], FP8, name="w1")
                nc.gpsimd.dma_start(w1_s[:], w_1.ap().rearrange("(ko p) n -> p ko n", p=P))
                w2_s = cst.tile([P, HID // P, C], FP8, name="w2")
                nc.gpsimd.dma_start(w2_s[:], w_2.ap().rearrange("(ko p) n -> p ko n", p=P))
                for rc in range(RS // 512):
                    for m in range(rc * 4, rc * 4 + 4):
                        ln_win(wk, ptp, x2_all[:, m, :], m)
                    for ht in range(HID // P):
                        ps = pps1.tile([P, 512], F32, tag="ps1")
                        for ko in range(0, C // P, 2):
                            nc.tensor.matmul(
                                ps[:], lhsT=w1_s[:, ko:ko + 2, ht * P:(ht + 1) * P],
                                rhs=zt_sbuf[:, ko:ko + 2, rc * 512:(rc + 1) * 512],
                                start=(ko == 0), stop=(ko == C // P - 2),
                                perf_mode=mybir.MatmulPerfMode.DoubleRow)
                        nc.scalar.activation(
                            hT_sbuf[:, ht, rc * 512:(rc + 1) * 512], ps[:],
                            AF.Gelu_apprx_tanh, bias=b1c_s[:, ht:ht + 1], scale=1.0 / WS)
                    for m in range(rc * 4, rc * 4 + 4):
                        ps = pps2.tile([P, C], F32, tag="ps2")
                        for ht in range(0, HID // P, 2):
                            nc.tensor.matmul(ps[:], lhsT=hT_sbuf[:, ht:ht + 2, m * P:(m + 1) * P],
                                             rhs=w2_s[:, ht:ht + 2, :],
                                             start=(ht == 0), stop=False,
                                             perf_mode=mybir.MatmulPerfMode.DoubleRow)
                        nc.tensor.matmul(ps[:], lhsT=ones_s[:], rhs=brow_s[0:1, 2 * C:3 * C],
                                         start=False, stop=True)
                        yt = wk.tile([P, C], F32, tag="y")
                        nc.vector.scalar_tensor_tensor(
                            out=yt[:], in0=ps[:], scalar=1.0 / WS, in1=x2_all[:, m, :],
                            op0=ALU.mult, op1=ALU.add)
                        nc.sync.dma_start(y_t[:, m, :], yt[:])

    nc.compile()
    return nc


def _prep(inputs):
    row = np.asarray(inputs["row_index"]).astype(np.int64).ravel()
    col = np.asarray(inputs["col_index"]).astype(np.int64).ravel()
    tcol = np.asarray(inputs["to_col_index"]).astype(np.int64).ravel()
    bias = np.asarray(inputs["pos_att_bias"], dtype=np.float32)
    dist = np.asarray(inputs["dist"], dtype=np.float32).ravel()
    pos = np.asarray(inputs["pos"], dtype=np.float32)
    cpos = np.asarray(inputs["col_pos"], dtype=np.float32)

    order = np.argsort(row, kind="stable")
    rs_, cs_, ts_ = row[order], col[order], tcol[order]
    win = rs_ // P
    counts = np.bincount(win, minlength=L // P)
    TW = int(np.ceil(max(int(counts.max()), 1) / P) * P)
    NT = TW // P
    starts = np.zeros(L // P + 1, np.int64)
    np.cumsum(counts, out=starts[1:])

    eidx_h = np.zeros((NCORES, NW, P, TW // 16), np.int16)
    rel4_h = np.zeros((NCORES, NW, P, NT, 4), np.float32)
    bias_h = np.full((NCORES, NW, P, NT, 8), -1e4, np.float32)
    s1_h = np.zeros((NCORES, NW, P, NT, P), np.float32)
    s2_h = np.zeros((NCORES, NW, P, NT, P), np.float32)

    for gw in range(L // P):
        c, w = divmod(gw, NW)
        s, e = int(starts[gw]), int(starts[gw + 1])
        n = e - s
        if n == 0:
            continue
        ecols = cs_[s:e]
        erows = (rs_[s:e] - gw * P).astype(np.int64)
        eo = order[s:e]
        j = np.arange(n)
        wrap = np.zeros((16, TW // 16), np.int16)
        wrap[j % 16, j // 16] = ecols.astype(np.int16)
        eidx_h[c, w] = np.tile(wrap, (8, 1))
        t_of = j // P
        e_of = j % P
        rel4_h[c, w, e_of, t_of, 0:3] = (cpos[ts_[s:e]] - pos[rs_[s:e]]) / dist[eo][:, None]
        rel4_h[c, w, e_of, t_of, 3] = 1.0
        bias_h[c, w, e_of, t_of, :] = bias[eo]
        s1_h[c, w, e_of, t_of, erows] = 1.0
        s2_h[c, w, erows, t_of, e_of] = 1.0

    import ml_dtypes
    bf = ml_dtypes.bfloat16
    return (TW, eidx_h, rel4_h.astype(bf), bias_h,
            s1_h.astype(bf), s2_h.astype(bf))


def kernel(**inputs):
    import ml_dtypes
    bf = ml_dtypes.bfloat16
    x = np.asarray(inputs["x"], dtype=np.float32)
    TW, eidx_h, rel4_h, bias_h, s1_h, s2_h = _prep(inputs)
    if TW not in _cache:
        _cache[TW] = _build(TW)
    nc = _cache[TW]

    f32 = lambda k: np.asarray(inputs[k], np.float32)
    g1, b1l = f32("ln1_g"), f32("ln1_b")
    g2, b2l = f32("ln2_g"), f32("ln2_b")
    Wq, Wk, Wv, Wo = f32("Wq"), f32("Wk"), f32("Wv"), f32("Wo")
    # Fold LN affine into the following matmuls; fold bk into bq (only the
    # difference q-k matters) and bv into bo (sum_e alpha = 1 per head).
    Wq_, Wk_, Wv_ = g1[:, None] * Wq, g1[:, None] * Wk, g1[:, None] * Wv
    bq_ = (b1l @ Wq + f32("bq")) - (b1l @ Wk + f32("bk"))
    bo_ = (b1l @ Wv + f32("bv")) @ Wo + f32("bo")
    W1_ = g2[:, None] * f32("W1")
    b1_ = b2l @ f32("W1") + f32("b1")
    import ml_dtypes as _md
    f8 = _md.float8_e4m3
    WS = 64.0
    w_qkv = (np.concatenate([Wq_, Wk_, Wv_], axis=1) * WS).astype(f8)

    wv4 = np.concatenate([f32("Wvec"), f32("bvec")[None, :]], axis=0)
    w_vec4 = np.zeros((32, C), np.float32)
    for h in range(H):
        w_vec4[4 * h:4 * h + 4, h * DH:(h + 1) * DH] = wv4[:, h * DH:(h + 1) * DH]

    brows = np.zeros((1, 4 * C), np.float32)
    brows[0, 0:C] = bq_
    brows[0, C:2 * C] = bo_
    brows[0, 2 * C:3 * C] = f32("b2")
    b1_col = np.ascontiguousarray(b1_.reshape(HID // P, P).T)

    in_maps = []
    for c in range(NCORES):
        in_maps.append(dict(
            x_in=np.ascontiguousarray(x[c * RS:(c + 1) * RS]),
            w_qkv=w_qkv, w_o=(Wo * WS).astype(f8),
            w_1=(W1_ * WS).astype(f8), w_2=(f32("W2") * WS).astype(f8),
            w_vec4=w_vec4.astype(bf), b1_col=b1_col,
            brows=(brows * WS).astype(f8),
            ident_in=np.eye(P, dtype=np.float32).astype(bf),
            ones_in=np.ones((1, P), np.float32).astype(f8),
            eps_in=np.full((P, 1), 1e-5, np.float32),
            eidx=eidx_h[c], rel4=rel4_h[c], biasC=bias_h[c],
            s_e2r=s1_h[c], s_r2e=s2_h[c],
        ))
    _last["nc"] = nc
    _last["in_maps"] = in_maps
    res = run_bass_kernel_spmd(nc, in_maps, list(range(NCORES)))
    global _last_res
    _last_res = res
    y = np.concatenate([res.results[c]["y_out"] for c in range(NCORES)], axis=0)
    return np.asarray(y, np.float32)


_last = {}
_last_res = None



# revision 66
# speedup vs baseline: 1.2112x; 1.2112x over previous
"""Trainium2 Bass kernel for nn_EncoderLayer_88476326298146 (sparse graph attention).

Row-sharded across 8 NeuronCores with host-side load balancing: all L rows are
LPT-packed into 128 bins (8 cores x 16 windows, exactly 128 rows each) so edge
counts per window are near-uniform; per-window tile counts (nts) are baked into
the build. k/v (k fp8 negated via host-negated Wk, v bf16; 1.5KB/row) are replicated via
AllGather; per-edge col features come from dma_gather on alternating SWDGE
queues (one gather fills a whole 1024-descriptor ring).

- LN affine folded into following weights host-side; biases via rank-1 ones-row
  matmuls on the PE.
- diff = q_row - k_col on the PE (s2^T@q then accumulate ident@(-k)); square on
  Act from PSUM; per-head reduce = two bf16 2x-mode halving adds + short reduce.
- exp emitted as bf16 PAIRS (eaux[...,0:16]) and shared by the alpha*v multiply
  (DVE 2x) and the aux (den/rel) matmul.
- segment softmax with m=0; segment sums via host-built one-hot matmuls.
- FFN1 produced transposed with gelu+bias fused on Act; FFN2 consumes h^T as
  lhsT. LN2 uses one batched Sqrt so the act table switches only once.
- DMA batching: s1+s2 in one tensor, rel+bias in one bf16 tensor, k+v in one
  store per window; ident/ones/eps generated on-chip.
"""
import os
import numpy as np

import concourse.bass as bass
import concourse.bacc as bacc
import concourse.mybir as mybir
import concourse.tile as tile
from concourse.bass_utils import run_bass_kernel_spmd
from concourse.library_config import mlp as mlp_lib

L, E, SP, C, H, DH, HID = 16384, 131072, 20000, 512, 8, 64, 1024
NCORES = 8
RS = L // NCORES
NW = RS // 128
P = 128
F32 = mybir.dt.float32
BF16 = mybir.dt.bfloat16
I16 = mybir.dt.int16
FP8 = mybir.dt.float8e4
U8 = mybir.dt.uint8
KVB = 3 * C  # kv row bytes: k fp8 (C) + v bf16 (2C)
WS = 64.0  # weight pre-scale (fp8 subnormal avoidance); descaled in Act casts
AF = mybir.ActivationFunctionType
ALU = mybir.AluOpType
AX = mybir.AxisListType

_cache = {}
_SKIP = set(os.environ.get("KSKIP", "").split(","))


def _build(nts):
    if isinstance(nts, int):
        nts = (nts // P,) * NW
    nts = tuple(int(n) for n in nts)
    assert len(nts) == NW
    NTmax = max(nts)
    TOT = sum(nts)             # total tiles across windows
    E16 = sum(n * P // 16 for n in nts)  # eidx columns
    toff = np.concatenate([[0], np.cumsum(nts)]).astype(int)
    GS = 2  # tiles per score group (PSUM: GS banks for qe)
    inv_s = 1.0 / float(np.sqrt(DH))
    nc = bacc.Bacc("TRN2", target_bir_lowering=False, debug=False, num_devices=NCORES,
                   num_swdge_queues=2)

    x_in = nc.dram_tensor("x_in", [RS, C], F32, kind="ExternalInput")
    w_qkv = nc.dram_tensor("w_qkv", [C, 3 * C], FP8, kind="ExternalInput")
    w_o = nc.dram_tensor("w_o", [C, C], FP8, kind="ExternalInput")
    w_1 = nc.dram_tensor("w_1", [C, HID], FP8, kind="ExternalInput")
    w_2 = nc.dram_tensor("w_2", [HID, C], FP8, kind="ExternalInput")
    w_vec4 = nc.dram_tensor("w_vec4", [32, C], BF16, kind="ExternalInput")
    b1_col = nc.dram_tensor("b1_col", [P, HID // P], F32, kind="ExternalInput")
    brows = nc.dram_tensor("brows", [1, 4 * C], FP8, kind="ExternalInput")
    eidx = nc.dram_tensor("eidx", [P, E16], I16, kind="ExternalInput")
    relbias = nc.dram_tensor("relbias", [P, TOT, 12], BF16, kind="ExternalInput")
    s12 = nc.dram_tensor("s12", [P, TOT, 2, P], BF16, kind="ExternalInput")
    y_out = nc.dram_tensor("y_out", [RS, C], F32, kind="ExternalOutput")

    x_t = x_in.ap().rearrange("(m p) n -> p m n", p=P)
    y_t = y_out.ap().rearrange("(m p) n -> p m n", p=P)

    with tile.TileContext(nc) as tc:
        with tc.tile_pool(name="dram", bufs=1, space="DRAM") as dram, \
             tc.tile_pool(name="const", bufs=1) as const, \
             tc.tile_pool(name="big", bufs=2) as big:
            nc.gpsimd.load_library(mlp_lib)

            # x loads first: LN(0) is the startup critical path.
            x_all = const.tile([P, NW, C], F32)
            nc.sync.dma_start(x_all[:, 0, :], x_t[:, 0, :])
            nc.sync.dma_start(x_all[:, 1, :], x_t[:, 1, :])
            nc.sync.dma_start(x_all[:, 2:4, :], x_t[:, 2:4, :])

            # on-chip constants: ident[p,j] = (j - p == 0), ones, eps (no DMAs
            # -> less HWDGE descriptor serialization at startup).
            iota_d = const.tile([P, P], I16)
            nc.gpsimd.iota(iota_d[:], pattern=[[1, P]], base=0, channel_multiplier=-1)
            ident = const.tile([P, P], BF16)
            nc.vector.tensor_scalar(out=ident[:], in0=iota_d[:], scalar1=0,
                                    scalar2=None, op0=ALU.is_equal)
            ident_f8 = const.tile([P, P], FP8)
            nc.vector.tensor_scalar(out=ident_f8[:], in0=iota_d[:], scalar1=0,
                                    scalar2=None, op0=ALU.is_equal)
            ones_s = const.tile([1, P], FP8)
            nc.vector.memset(ones_s[:], 1.0)
            eps_t = const.tile([P, 1], F32)
            nc.vector.memset(eps_t[:], 1e-5)


            brow_s = const.tile([1, 4 * C], FP8)
            nc.sync.dma_start(brow_s[:], brows.ap())
            wvec_s = const.tile([32, C], BF16)
            nc.sync.dma_start(wvec_s[:], w_vec4.ap())
            b1c_s = const.tile([P, HID // P], F32)
            nc.sync.dma_start(b1c_s[:], b1_col.ap())

            # weight prefetch (Pool queue; overlaps P1)
            wqkv_s = const.tile([P, C // P, 3 * C], FP8, name="wqkv")
            nc.gpsimd.dma_start(wqkv_s[:], w_qkv.ap().rearrange("(ko p) n -> p ko n", p=P))
            wo_s = const.tile([P, C // P, C], FP8, name="wo")
            nc.gpsimd.dma_start(wo_s[:], w_o.ap().rearrange("(ko p) n -> p ko n", p=P))
            w1_s = const.tile([P, C // P, HID], FP8, name="w1")
            nc.gpsimd.dma_start(w1_s[:], w_1.ap().rearrange("(ko p) n -> p ko n", p=P))
            w2_s = const.tile([P, HID // P, C], FP8, name="w2")
            nc.gpsimd.dma_start(w2_s[:], w_2.ap().rearrange("(ko p) n -> p ko n", p=P))
            nc.sync.dma_start(x_all[:, 4:8, :], x_t[:, 4:8, :])
            nc.sync.dma_start(x_all[:, 8:12, :], x_t[:, 8:12, :])
            nc.sync.dma_start(x_all[:, 12:16, :], x_t[:, 12:16, :])

            x2_all = const.tile([P, NW, C], BF16)
            sx_all = const.tile([P, NW], F32)    # per-window sum(x2) (LN2)
            sx2_all = const.tile([P, NW], F32)   # per-window sum(x2^2)
            zt_sbuf = const.tile([P, C // P, RS], FP8)
            q_sbuf = const.tile([P, NW, C], BF16)
            hT_sbuf = const.tile([P, HID // P, RS], FP8)

            kv_shard = dram.tile([RS, KVB], U8)
            if "ag" not in _SKIP:
                kvt = dram.tile([L, KVB], U8, addr_space="Shared")
            else:
                kvt = dram.tile([L, KVB], U8)

            # Edge-phase loads for the first windows issued BEFORE P1 so they
            # prefetch during P1 (the SP ring is in-order; emitting them after
            # P1's kv stores would delay them to the end of P1).
            edge_tiles = {}
            for w in range(3):
                NT = nts[w]
                to = int(toff[w])
                s12_t = big.tile([P, NTmax, 2, P], BF16, tag="s12", bufs=3)
                nc.sync.dma_start(s12_t[:, 0:NT, :, :],
                                  s12.ap()[:, to:to + NT, :, :])
                rb_t = big.tile([P, NTmax, 12], BF16, tag="rb", bufs=3)
                nc.sync.dma_start(rb_t[:, 0:NT, :],
                                  relbias.ap()[:, to:to + NT, :])
                edge_tiles[w] = (s12_t, rb_t)

            # ---------- LN helper: stats + normalized bf16 z (no affine) ----
            def ln_win(wk, ptp, src, m, copy_eng="v"):
                stats = wk.tile([P, 6], F32, tag="ln_st")
                nc.vector.bn_stats(stats[:], src)
                mv = wk.tile([P, 2], F32, tag="ln_mv")
                nc.vector.bn_aggr(mv[:], stats[:])
                sd = wk.tile([P, 1], F32, tag="ln_sd")
                nc.scalar.activation(sd[:], mv[:, 1:2], AF.Sqrt, bias=eps_t[:], scale=1.0)
                rs_ = wk.tile([P, 1], F32, tag="ln_rs")
                nc.vector.reciprocal(rs_[:], sd[:])
                nmr = wk.tile([P, 1], F32, tag="ln_nmr")
                nc.vector.scalar_tensor_tensor(
                    out=nmr[:], in0=mv[:, 0:1], scalar=-1.0, in1=rs_[:],
                    op0=ALU.mult, op1=ALU.mult)
                zf = wk.tile([P, C], BF16, tag="ln_zf")
                nc.scalar.activation(zf[:], src, AF.Identity, bias=nmr[:], scale=rs_[:])
                tp = ptp.tile([P, C], BF16, tag="tp")
                for c4 in range(C // P):
                    nc.tensor.transpose(tp[:, c4 * P:(c4 + 1) * P],
                                        zf[:, c4 * P:(c4 + 1) * P], ident[:])
                if copy_eng == "a":
                    nc.scalar.activation(
                        zt_sbuf[:, :, m * P:(m + 1) * P],
                        tp[:].rearrange("p (c4 q) -> p c4 q", q=P), AF.Identity)
                elif copy_eng == "g":
                    nc.gpsimd.tensor_copy(
                        zt_sbuf[:, :, m * P:(m + 1) * P],
                        tp[:].rearrange("p (c4 q) -> p c4 q", q=P))
                else:
                    nc.vector.tensor_copy(
                        zt_sbuf[:, :, m * P:(m + 1) * P],
                        tp[:].rearrange("p (c4 q) -> p c4 q", q=P))

            # ============ P1: LN1 + QKV ============
            if "p1" not in _SKIP:
             with tc.tile_pool(name="p1", bufs=4) as wk, \
                 tc.tile_pool(name="p1tp", bufs=4, space="PSUM") as ptp, \
                 tc.tile_pool(name="p1ps", bufs=4, space="PSUM") as pps:
                kv_sh_t = kv_shard[:].rearrange("(m p) n -> p m n", p=P)
                for m in range(NW):
                    # zt copies alternate Act/DVE; nothing from P1 runs on the
                    # Pool queue, so gather(0)'s descriptor-gen is not blocked
                    # behind P1 (Pool is in-order).
                    ln_win(wk, ptp, x_all[:, m, :], m,
                           copy_eng=("a" if m % 2 == 0 else "v"))
                    kvb = wk.tile([P, KVB], U8, tag="kvb")
                    for nb in range(3):
                        ps = pps.tile([P, C], F32, tag="ps")
                        for ko in range(0, C // P, 2):
                            nc.tensor.matmul(
                                ps[:],
                                lhsT=zt_sbuf[:, ko:ko + 2, m * P:(m + 1) * P],
                                rhs=wqkv_s[:, ko:ko + 2, nb * C:(nb + 1) * C],
                                start=(ko == 0), stop=(ko == C // P - 2 and nb != 0),
                                perf_mode=mybir.MatmulPerfMode.DoubleRow)
                        if nb == 0:
                            nc.tensor.matmul(ps[:], lhsT=ones_s[:], rhs=brow_s[0:1, 0:C],
                                             start=False, stop=True)
                            nc.scalar.activation(q_sbuf[:, m, :], ps[:], AF.Identity,
                                                 scale=1.0 / WS)
                        else:
                            # k stored negated (Wk negated host-side) so the edge
                            # phase accumulates qe + (-k) on the PE via ident.
                            # k cast on Act, v on DVE; one combined DMA.
                            if nb == 1:
                                nc.scalar.mul(kvb[:, 0:C].bitcast(FP8), ps[:], 1.0 / WS)
                            else:
                                nc.vector.tensor_scalar_mul(
                                    kvb[:, C:KVB].bitcast(BF16), ps[:], 1.0 / WS)
                    nc.sync.dma_start(kv_sh_t[:, m, :], kvb[:])

            # ============ P2: AllGather ============
            if "ag" not in _SKIP:
                nc.gpsimd.collective_compute(
                    "AllGather", ALU.bypass, replica_groups=[list(range(NCORES))],
                    ins=[kv_shard[:].opt()], outs=[kvt[:].opt()])

            # ============ P3: edge windows + Wo + residual ============
            # `big` lives at top level so s12/idx DMAs and gathers are not
            # WAR-serialized behind P1's SBUF.
            if "edge" not in _SKIP:
             with tc.tile_pool(name="ew", bufs=4) as wk, \
                 tc.tile_pool(name="pqe", bufs=1, space="PSUM") as pqe, \
                 tc.tile_pool(name="ppsV", bufs=2, space="PSUM") as ppsV, \
                 tc.tile_pool(name="ppsA", bufs=2, space="PSUM") as ppsA, \
                 tc.tile_pool(name="ptpc", bufs=1, space="PSUM") as ptpc, \
                 tc.tile_pool(name="p5ps", bufs=1, space="PSUM") as p5ps:
                # tail(w): den -> rden -> anr -> an_ts -> wvec-mm -> attin ->
                # transpose -> Wo-mm -> x2.  Emitted one window late (split in
                # two parts interleaved with window w+1's groups) so its long
                # cross-engine latency chain overlaps the next window's bulk
                # work instead of stalling the in-order engine queues.
                def tail_a(w, psV, psA):
                    den = wk.tile([P, 16], F32, tag="den")
                    nc.vector.tensor_scalar_max(den[:], psA[:, 0:16], 1e-30)
                    rden = wk.tile([P, 16], F32, tag="rden")
                    nc.vector.reciprocal(rden[:], den[:])
                    # fold the (unnormalized) Wvec term into psV: w_vec4 is
                    # head-block-diagonal, so per-(row,head) rden factors
                    # through the sum.
                    anr = wk.tile([P, 32], BF16, tag="anr")
                    nc.scalar.activation(anr[:], psA[:, 16:48], AF.Identity)
                    tpc = ptpc.tile([P, C], BF16, tag="tpc")
                    nc.tensor.transpose(tpc[0:32, 0:P], anr[:], ident[:])
                    an_ts = wk.tile([32, P], BF16, tag="an_ts")
                    nc.scalar.activation(an_ts[:], tpc[0:32, 0:P], AF.Identity)
                    nc.tensor.matmul(psV[:], lhsT=an_ts[:], rhs=wvec_s[:],
                                     start=False, stop=True)
                    attin = wk.tile([P, C], BF16, tag="attin")
                    nc.vector.tensor_mul(
                        attin[:].rearrange("p (h d) -> p h d", h=H),
                        psV[:].rearrange("p (h d) -> p h d", h=H),
                        rden[:].rearrange("p (h j) -> p h j", h=H)[:, :, 0:1]
                            .broadcast_to([P, H, DH]))
                    return attin

                def tail_b(w, attin):
                    tpa = ptpc.tile([P, C], BF16, tag="tpc")
                    for c4 in range(C // P):
                        nc.tensor.transpose(tpa[:, c4 * P:(c4 + 1) * P],
                                            attin[:, c4 * P:(c4 + 1) * P], ident[:])
                    at_sb = wk.tile([P, C // P, P], FP8, tag="at_sb")
                    nc.scalar.activation(
                        at_sb[:], tpa[:].rearrange("p (c4 q) -> p c4 q", q=P),
                        AF.Identity)
                    x2ps = p5ps.tile([P, C], F32, tag="p5")
                    for ko in range(0, C // P, 2):
                        nc.tensor.matmul(x2ps[:], lhsT=at_sb[:, ko:ko + 2, :],
                                         rhs=wo_s[:, ko:ko + 2, :],
                                         start=(ko == 0), stop=False,
                                         perf_mode=mybir.MatmulPerfMode.DoubleRow)
                    nc.tensor.matmul(x2ps[:], lhsT=ones_s[:], rhs=brow_s[0:1, C:2 * C],
                                     start=False, stop=True)
                    nc.vector.scalar_tensor_tensor(
                        out=x2_all[:, w, :], in0=x2ps[:], scalar=1.0 / WS,
                        in1=x_all[:, w, :], op0=ALU.mult, op1=ALU.add)
                    # LN2 stats via the Act accumulator (Act has slack in the
                    # edge phase; keeps the bn_stats chain out of the tail).
                    trash = wk.tile([P, C], BF16, tag="trash")
                    nc.scalar.activation(trash[:], x2_all[:, w, :], AF.Square,
                                         accum_out=sx2_all[:, w:w + 1])
                    nc.scalar.activation(trash[:], x2_all[:, w, :], AF.Identity,
                                         accum_out=sx_all[:, w:w + 1])

                prev = None       # (w, psV, psA) of the previous window
                prev_attin = None  # (w, attin) pending tail_b
                for w in range(NW):
                    NT = nts[w]
                    TW = NT * P
                    eo = sum(n * P // 16 for n in nts[:w])
                    to = int(toff[w])
                    if w in edge_tiles:
                        s12_t, rb_t = edge_tiles.pop(w)
                    else:
                        s12_t = big.tile([P, NTmax, 2, P], BF16, tag="s12", bufs=3)
                        nc.sync.dma_start(s12_t[:, 0:2, :, :],
                                          s12.ap()[:, to:to + 2, :, :])
                        nc.sync.dma_start(s12_t[:, 2:NT, :, :],
                                          s12.ap()[:, to + 2:to + NT, :, :])
                        rb_t = big.tile([P, NTmax, 12], BF16, tag="rb", bufs=3)
                        nc.sync.dma_start(rb_t[:, 0:NT, :],
                                          relbias.ap()[:, to:to + NT, :])
                    idx_t = big.tile([P, NTmax * P // 16], I16, tag="idx", bufs=3)
                    nc.sync.dma_start(idx_t[:, 0:TW // 16],
                                      eidx.ap()[:, eo:eo + TW // 16])
                    kv_g = big.tile([P, NTmax, KVB], U8, tag="kv", bufs=3)
                    # split each gather into quarter-gathers alternating the
                    # two SWDGE queues: the first chunk lands earlier (qe for
                    # the first tiles starts sooner) and rings stay pipelined.
                    QC = 1  # tiles per gather chunk
                    for ci, c0 in enumerate(range(0, NT, QC)):
                        cb = min(QC, NT - c0)
                        nc.gpsimd.dma_gather(
                            out_ap=kv_g[:, c0:c0 + cb, :], in_ap=kvt[:],
                            idxs_ap=idx_t[:, c0 * P // 16:(c0 + cb) * P // 16],
                            num_idxs=cb * P, num_idxs_reg=cb * P, elem_size=KVB,
                            single_packet=False, queue_num=ci % 2)

                    psV = ppsV.tile([P, 512], F32, tag="psV")
                    psA = ppsA.tile([P, 48], F32, tag="psA")
                    ngrp = (NT + GS - 1) // GS
                    for gi, t0 in enumerate(range(0, NT, GS)):
                        tb = min(GS, NT - t0)
                        qe = pqe.tile([P, GS, C], F32, tag="qe")
                        for d_ in range(tb):
                            nc.tensor.matmul(qe[:, d_, :],
                                             lhsT=s12_t[:, t0 + d_, 1, :],
                                             rhs=q_sbuf[:, w, :], start=True, stop=False)
                            nc.tensor.matmul(qe[:, d_, :], lhsT=ident_f8[:],
                                             rhs=kv_g[:, t0 + d_, 0:C].bitcast(FP8),
                                             start=False, stop=True)
                        if gi == 1 and prev is not None:
                            if prev_attin is not None:
                                tail_b(*prev_attin)
                            prev_attin = (prev[0], tail_a(*prev))
                            prev = None
                        dsq = wk.tile([P, GS, C], BF16, tag="dsq")
                        nc.scalar.activation(dsq[:, 0:tb, :], qe[:, 0:tb, :], AF.Square)
                        # staged-halving reduce: two bf16 2x-mode adds, then a
                        # short TensorReduce (TensorReduce has no fast mode).
                        d4 = dsq[:, 0:tb, :].rearrange("p t (h j d) -> p t h j d", h=H, j=2)
                        h1 = wk.tile([P, GS, H, DH // 2], BF16, tag="h1")
                        nc.vector.tensor_add(h1[:, 0:tb, :, :], d4[:, :, :, 0, :], d4[:, :, :, 1, :])
                        h14 = h1[:, 0:tb, :, :].rearrange("p t h (j d) -> p t h j d", j=2)
                        h2 = wk.tile([P, GS, H, DH // 4], BF16, tag="h2")
                        nc.vector.tensor_add(h2[:, 0:tb, :, :], h14[:, :, :, 0, :], h14[:, :, :, 1, :])
                        s8 = wk.tile([P, GS, H], F32, tag="s8")
                        nc.vector.reduce_sum(
                            s8[:, 0:tb, :], h2[:, 0:tb, :, :], axis=AX.X)
                        sc = wk.tile([P, GS, H], F32, tag="sc")
                        nc.vector.scalar_tensor_tensor(
                            out=sc[:, 0:tb, :], in0=s8[:, 0:tb, :], scalar=-inv_s,
                            in1=rb_t[:, t0:t0 + tb, 4:12], op0=ALU.mult, op1=ALU.add)
                        eaux = wk.tile([P, GS, 48], BF16, tag="eaux")
                        nc.scalar.activation(
                            eaux[:, 0:tb, 0:16].rearrange("p t (h j) -> p t h j", h=H),
                            sc[:, 0:tb, :].unsqueeze(3).broadcast_to([P, tb, H, 2]),
                            AF.Exp)
                        pev = wk.tile([P, GS, C], BF16, tag="pev")
                        for d_ in range(tb):
                            nc.vector.tensor_mul(
                                pev[:, d_, :].rearrange("p (h a j) -> p h a j", h=H, j=2),
                                eaux[:, d_, 0:16].rearrange("p (h j) -> p h j", h=H)
                                    .unsqueeze(2).broadcast_to([P, H, DH // 2, 2]),
                                kv_g[:, t0 + d_, C:KVB].bitcast(BF16)
                                    .rearrange("p (h a j) -> p h a j", h=H, j=2))
                        nc.vector.tensor_mul(
                            eaux[:, 0:tb, 16:48].rearrange("p t (h a) -> p t h a", h=H),
                            eaux[:, 0:tb, 0:16].rearrange("p t (h j) -> p t h j", h=H)[:, :, :, 0:1]
                                .broadcast_to([P, tb, H, 4]),
                            rb_t[:, t0:t0 + tb, 0:4].unsqueeze(2)
                                .broadcast_to([P, tb, H, 4]))
                        for d_ in range(tb):
                            t = t0 + d_
                            nc.tensor.matmul(psV[:], lhsT=s12_t[:, t, 0, :],
                                             rhs=pev[:, d_, :],
                                             start=(t == 0), stop=False)
                            nc.tensor.matmul(psA[:], lhsT=s12_t[:, t, 0, :],
                                             rhs=eaux[:, d_, :],
                                             start=(t == 0), stop=(t == NT - 1))
                        if gi == 2 and prev_attin is not None:
                            tail_b(*prev_attin)
                            prev_attin = None
                    prev = (w, psV, psA)
                if prev is not None:
                    prev_attin = (prev[0], tail_a(*prev))
                if prev_attin is not None:
                    tail_b(*prev_attin)

            # ============ P4: LN2 + FFN ============
            if "p4" not in _SKIP:
             with tc.tile_pool(name="p4", bufs=4) as wk, \
                 tc.tile_pool(name="p4tp", bufs=4, space="PSUM") as ptp, \
                 tc.tile_pool(name="f1ps", bufs=2, space="PSUM") as pps1, \
                 tc.tile_pool(name="f2ps", bufs=2, space="PSUM") as pps2:
                # mean/var from the Act-accumulated sums: mean = sx/C,
                # var = sx2/C - mean^2; one batched Sqrt (one table switch).
                mean_all = wk.tile([P, NW], F32, tag="mean_all")
                nc.vector.tensor_scalar_mul(mean_all[:], sx_all[:], 1.0 / C)
                msq = wk.tile([P, NW], F32, tag="msq")
                nc.vector.tensor_mul(msq[:], mean_all[:], mean_all[:])
                var_all = wk.tile([P, NW], F32, tag="var_all")
                nc.vector.scalar_tensor_tensor(
                    out=var_all[:], in0=sx2_all[:], scalar=1.0 / C,
                    in1=msq[:], op0=ALU.mult, op1=ALU.subtract)
                sd_all = wk.tile([P, NW], F32, tag="sd_all")
                nc.scalar.activation(sd_all[:], var_all[:], AF.Sqrt,
                                     bias=eps_t[:], scale=1.0)
                rs_all = wk.tile([P, NW], F32, tag="rs_all")
                nc.vector.reciprocal(rs_all[:], sd_all[:])
                nmr_all = wk.tile([P, NW], F32, tag="nmr_all")
                nc.vector.scalar_tensor_tensor(
                    out=nmr_all[:], in0=mean_all[:], scalar=-1.0,
                    in1=rs_all[:], op0=ALU.mult, op1=ALU.mult)
                for m in range(NW):
                    # zf on DVE (2x mode, per-partition AP scalars): frees Act
                    # for the gelus.
                    zf = wk.tile([P, C], BF16, tag="ln_zf")
                    nc.vector.tensor_scalar(
                        out=zf[:], in0=x2_all[:, m, :],
                        scalar1=rs_all[:, m:m + 1], scalar2=nmr_all[:, m:m + 1],
                        op0=ALU.mult, op1=ALU.add)
                    tp = ptp.tile([P, C], BF16, tag="tp")
                    for c4 in range(C // P):
                        nc.tensor.transpose(tp[:, c4 * P:(c4 + 1) * P],
                                            zf[:, c4 * P:(c4 + 1) * P], ident[:])
                    if m % 2 == 0:
                        nc.scalar.activation(
                            zt_sbuf[:, :, m * P:(m + 1) * P],
                            tp[:].rearrange("p (c4 q) -> p c4 q", q=P), AF.Identity)
                    else:
                        nc.vector.tensor_copy(
                            zt_sbuf[:, :, m * P:(m + 1) * P],
                            tp[:].rearrange("p (c4 q) -> p c4 q", q=P))
                for rc in range(RS // 512):
                    for ht in range(HID // P):
                        ps = pps1.tile([P, 512], F32, tag="ps1")
                        for ko in range(0, C // P, 2):
                            nc.tensor.matmul(
                                ps[:], lhsT=w1_s[:, ko:ko + 2, ht * P:(ht + 1) * P],
                                rhs=zt_sbuf[:, ko:ko + 2, rc * 512:(rc + 1) * 512],
                                start=(ko == 0), stop=(ko == C // P - 2),
                                perf_mode=mybir.MatmulPerfMode.DoubleRow)
                        nc.scalar.activation(
                            hT_sbuf[:, ht, rc * 512:(rc + 1) * 512], ps[:],
                            AF.Gelu_apprx_tanh, bias=b1c_s[:, ht:ht + 1], scale=1.0 / WS)
                    for m in range(rc * 4, rc * 4 + 4):
                        ps = pps2.tile([P, C], F32, tag="ps2")
                        for ht in range(0, HID // P, 2):
                            nc.tensor.matmul(ps[:], lhsT=hT_sbuf[:, ht:ht + 2, m * P:(m + 1) * P],
                                             rhs=w2_s[:, ht:ht + 2, :],
                                             start=(ht == 0), stop=False,
                                             perf_mode=mybir.MatmulPerfMode.DoubleRow)
                        nc.tensor.matmul(ps[:], lhsT=ones_s[:], rhs=brow_s[0:1, 2 * C:3 * C],
                                         start=False, stop=True)
                        if m % 2 == 0:
                            yt = wk.tile([P, 2, C], F32, tag="y", bufs=2)
                        nc.vector.scalar_tensor_tensor(
                            out=yt[:, m % 2, :], in0=ps[:], scalar=1.0 / WS,
                            in1=x2_all[:, m, :], op0=ALU.mult, op1=ALU.add)
                        if m % 2 == 1:
                            nc.sync.dma_start(y_t[:, m - 1:m + 1, :], yt[:])

    nc.compile()
    return nc


def _prep(inputs):
    row = np.asarray(inputs["row_index"]).astype(np.int64).ravel()
    col = np.asarray(inputs["col_index"]).astype(np.int64).ravel()
    tcol = np.asarray(inputs["to_col_index"]).astype(np.int64).ravel()
    bias = np.asarray(inputs["pos_att_bias"], dtype=np.float32)
    dist = np.asarray(inputs["dist"], dtype=np.float32).ravel()
    pos = np.asarray(inputs["pos"], dtype=np.float32)
    cpos = np.asarray(inputs["col_pos"], dtype=np.float32)

    # ---- balance rows into 128 bins (8 cores x 16 windows, 128 rows each) ---
    import heapq
    NB = NCORES * NW
    cnt = np.bincount(row, minlength=L)
    order_r = np.argsort(-cnt, kind="stable")
    heap = [(0, 0, b) for b in range(NB)]
    heapq.heapify(heap)
    bin_rows = [[] for _ in range(NB)]
    bin_sum = np.zeros(NB, np.int64)
    for r in order_r:
        popped = []
        while True:
            s, n, b = heapq.heappop(heap)
            if n < P:
                break
            popped.append((s, n, b))
        for x_ in popped:
            heapq.heappush(heap, x_)
        bin_rows[b].append(int(r))
        bin_sum[b] = s + cnt[r]
        heapq.heappush(heap, (int(bin_sum[b]), n + 1, b))

    # snake-assign bins to cores by edge-count rank; windows sorted descending
    # within each core so window j's count is similar across cores.
    rk = np.argsort(-bin_sum)
    core_bins = [[] for _ in range(NCORES)]
    for i, b in enumerate(rk):
        core_bins[i % NCORES].append(int(b))
    for c in range(NCORES):
        core_bins[c].sort(key=lambda b: -int(bin_sum[b]))
    counts = np.array([[bin_sum[b] for b in core_bins[c]] for c in range(NCORES)])
    nts = tuple(int(x) for x in np.ceil(counts.max(axis=0) / P).astype(int))

    # global row permutation: new row (c*RS + w*P + slot) = old row
    perm = np.empty(L, np.int64)
    for c in range(NCORES):
        for w in range(NW):
            b = core_bins[c][w]
            perm[c * RS + w * P:c * RS + (w + 1) * P] = bin_rows[b]
    inv_perm = np.empty(L, np.int64)
    inv_perm[perm] = np.arange(L)

    new_row = inv_perm[row]   # position of each edge's target row
    new_col = inv_perm[col]   # position of each edge's source col in permuted kvt

    TOT = sum(nts)
    E16 = sum(n * P // 16 for n in nts)
    toff = np.concatenate([[0], np.cumsum(nts)]).astype(int)
    eoff = np.concatenate([[0], np.cumsum([n * P // 16 for n in nts])]).astype(int)

    eidx_h = np.zeros((NCORES, P, E16), np.int16)
    rb_h = np.zeros((NCORES, P, TOT, 12), np.float32)
    rb_h[:, :, :, 4:12] = -1e4
    s12_h = np.zeros((NCORES, P, TOT, 2, P), np.float32)

    gw_all = new_row // P  # global window id (0..127) per edge
    order_e = np.argsort(gw_all, kind="stable")
    gw_s = gw_all[order_e]
    starts = np.searchsorted(gw_s, np.arange(NB + 1))
    for gw in range(NB):
        c, w = divmod(gw, NW)
        sl = order_e[starts[gw]:starts[gw + 1]]
        n = len(sl)
        if n == 0:
            continue
        TWw = nts[w] * P
        assert n <= TWw, (n, TWw)
        erows = (new_row[sl] - gw * P).astype(np.int64)
        ecols = new_col[sl]
        j = np.arange(n)
        wrap = np.zeros((16, TWw // 16), np.int16)
        wrap[j % 16, j // 16] = ecols.astype(np.int16)
        eidx_h[c, :, eoff[w]:eoff[w + 1]] = np.tile(wrap, (8, 1))
        t_of = toff[w] + j // P
        e_of = j % P
        rb_h[c, e_of, t_of, 0:3] = (cpos[tcol[sl]] - pos[row[sl]]) / dist[sl][:, None]
        rb_h[c, e_of, t_of, 3] = 1.0
        rb_h[c, e_of, t_of, 4:12] = bias[sl]
        s12_h[c, e_of, t_of, 0, erows] = 1.0   # s1: edge -> row scatter
        s12_h[c, erows, t_of, 1, e_of] = 1.0   # s2: row -> edge expand
    import ml_dtypes
    bf = ml_dtypes.bfloat16
    return nts, perm, eidx_h, rb_h.astype(bf), s12_h.astype(bf)


def kernel(**inputs):
    import ml_dtypes
    bf = ml_dtypes.bfloat16
    x = np.asarray(inputs["x"], dtype=np.float32)
    nts, perm, eidx_h, rb_h, s12_h = _prep(inputs)
    if nts not in _cache:
        _cache[nts] = _build(nts)
    nc = _cache[nts]

    f32 = lambda k: np.asarray(inputs[k], np.float32)
    g1, b1l = f32("ln1_g"), f32("ln1_b")
    g2, b2l = f32("ln2_g"), f32("ln2_b")
    Wq, Wk, Wv, Wo = f32("Wq"), f32("Wk"), f32("Wv"), f32("Wo")
    # Fold LN affine into the following matmuls; fold bk into bq (only the
    # difference q-k matters) and bv into bo (sum_e alpha = 1 per head).
    Wq_, Wk_, Wv_ = g1[:, None] * Wq, g1[:, None] * Wk, g1[:, None] * Wv
    bq_ = (b1l @ Wq + f32("bq")) - (b1l @ Wk + f32("bk"))
    bo_ = (b1l @ Wv + f32("bv")) @ Wo + f32("bo")
    W1_ = g2[:, None] * f32("W1")
    b1_ = b2l @ f32("W1") + f32("b1")
    import ml_dtypes as _md
    f8 = _md.float8_e4m3
    WS = 64.0
    # Wk negated: the kernel stores k pre-negated for the PE qe-k accumulate.
    w_qkv = (np.concatenate([Wq_, -Wk_, Wv_], axis=1) * WS).astype(f8)

    wv4 = np.concatenate([f32("Wvec"), f32("bvec")[None, :]], axis=0)
    w_vec4 = np.zeros((32, C), np.float32)
    for h in range(H):
        w_vec4[4 * h:4 * h + 4, h * DH:(h + 1) * DH] = wv4[:, h * DH:(h + 1) * DH]

    brows = np.zeros((1, 4 * C), np.float32)
    brows[0, 0:C] = bq_
    brows[0, C:2 * C] = bo_
    brows[0, 2 * C:3 * C] = f32("b2")
    b1_col = np.ascontiguousarray(b1_.reshape(HID // P, P).T)

    xp = x[perm]
    in_maps = []
    for c in range(NCORES):
        in_maps.append(dict(
            x_in=np.ascontiguousarray(xp[c * RS:(c + 1) * RS]),
            w_qkv=w_qkv, w_o=(Wo * WS).astype(f8),
            w_1=(W1_ * WS).astype(f8), w_2=(f32("W2") * WS).astype(f8),
            w_vec4=w_vec4.astype(bf), b1_col=b1_col,
            brows=(brows * WS).astype(f8),
            eidx=eidx_h[c], relbias=rb_h[c], s12=s12_h[c],
        ))
    _last["nc"] = nc
    _last["in_maps"] = in_maps
    res = run_bass_kernel_spmd(nc, in_maps, list(range(NCORES)))
    global _last_res
    _last_res = res
    yp = np.concatenate([res.results[c]["y_out"] for c in range(NCORES)], axis=0)
    y = np.empty_like(yp)
    y[perm] = yp
    return np.asarray(y, np.float32)


_last = {}
_last_res = None


# revision 79
# speedup vs baseline: 1.2725x; 1.0507x over previous
"""Trainium2 Bass kernel for nn_EncoderLayer_88476326298146 (sparse graph attention).

Row-sharded across 8 NeuronCores with host-side load balancing: all L rows are
LPT-packed into 128 bins (8 cores x 16 windows, exactly 128 rows each) so edge
counts per window are near-uniform; per-window tile counts (nts) are baked into
the build. k/v (k fp8 negated via host-negated Wk, v bf16; 1.5KB/row) are replicated via
AllGather; per-edge col features come from per-tile dma_gather chunks
alternating the two SWDGE queues (early chunks land sooner, and one full-window
gather would fill a whole 1024-descriptor ring).

- LN affine folded into following weights host-side; biases via rank-1 ones-row
  matmuls on the PE.
- diff = q_row - k_col on the PE (s2^T@q then accumulate ident@(-k)); square on
  Act from PSUM; per-head reduce = two bf16 2x-mode halving adds + short reduce.
- exp emitted as bf16 PAIRS (eaux[...,0:16]) and shared by the alpha*v multiply
  (DVE 2x) and the aux (den/rel) matmul.
- segment softmax with m=0; segment sums via host-built one-hot matmuls.
- FFN1 produced transposed with gelu+bias fused on Act; FFN2 consumes h^T as
  lhsT. LN2 uses one batched Sqrt so the act table switches only once.
- DMA batching: s1+s2 in one tensor, rel+bias in one bf16 tensor, k+v in one
  store per window; ident/ones/eps generated on-chip.
"""
import os
import numpy as np

import concourse.bass as bass
import concourse.bacc as bacc
import concourse.mybir as mybir
import concourse.tile as tile
from concourse.bass_utils import run_bass_kernel_spmd
from concourse.library_config import mlp as mlp_lib

L, E, SP, C, H, DH, HID = 16384, 131072, 20000, 512, 8, 64, 1024
NCORES = 8
RS = L // NCORES
NW = RS // 128
P = 128
F32 = mybir.dt.float32
BF16 = mybir.dt.bfloat16
I16 = mybir.dt.int16
FP8 = mybir.dt.float8e4
U8 = mybir.dt.uint8
KVB = 3 * C  # kv row bytes: k fp8 (C) + v bf16 (2C)
WS = 64.0  # weight pre-scale (fp8 subnormal avoidance); descaled in Act casts
AF = mybir.ActivationFunctionType
ALU = mybir.AluOpType
AX = mybir.AxisListType

_cache = {}
_SKIP = set(os.environ.get("KSKIP", "").split(","))


def _build(nts):
    if isinstance(nts, int):
        nts = (nts // P,) * NW
    nts = tuple(int(n) for n in nts)
    assert len(nts) == NW
    NTmax = max(nts)
    TOT = sum(nts)             # total tiles across windows
    E16 = sum(n * P // 16 for n in nts)  # eidx columns
    toff = np.concatenate([[0], np.cumsum(nts)]).astype(int)
    GS = 2  # tiles per score group (PSUM: GS banks for qe)
    inv_s = 1.0 / float(np.sqrt(DH))
    nc = bacc.Bacc("TRN2", target_bir_lowering=False, debug=False, num_devices=NCORES,
                   num_swdge_queues=2)

    x_in = nc.dram_tensor("x_in", [RS, C], F32, kind="ExternalInput")
    w_qkv = nc.dram_tensor("w_qkv", [C, 3 * C], FP8, kind="ExternalInput")
    w_o = nc.dram_tensor("w_o", [C, C], FP8, kind="ExternalInput")
    w_1 = nc.dram_tensor("w_1", [C, HID], FP8, kind="ExternalInput")
    w_2 = nc.dram_tensor("w_2", [HID, C], FP8, kind="ExternalInput")
    w_vec4 = nc.dram_tensor("w_vec4", [32, C], BF16, kind="ExternalInput")
    b1_col = nc.dram_tensor("b1_col", [P, HID // P], F32, kind="ExternalInput")
    brows = nc.dram_tensor("brows", [1, 4 * C], FP8, kind="ExternalInput")
    eidx = nc.dram_tensor("eidx", [P, E16], I16, kind="ExternalInput")
    relbias = nc.dram_tensor("relbias", [P, TOT, 12], BF16, kind="ExternalInput")
    s12 = nc.dram_tensor("s12", [P, TOT, 2, P], BF16, kind="ExternalInput")
    y_out = nc.dram_tensor("y_out", [RS, C], F32, kind="ExternalOutput")

    x_t = x_in.ap().rearrange("(m p) n -> p m n", p=P)
    y_t = y_out.ap().rearrange("(m p) n -> p m n", p=P)

    with tile.TileContext(nc) as tc:
        with tc.tile_pool(name="dram", bufs=1, space="DRAM") as dram, \
             tc.tile_pool(name="const", bufs=1) as const, \
             tc.tile_pool(name="big", bufs=2) as big:
            nc.gpsimd.load_library(mlp_lib)

            # x loads first: LN(0) is the startup critical path.
            x_all = const.tile([P, NW, C], F32)
            nc.sync.dma_start(x_all[:, 0, :], x_t[:, 0, :])
            nc.sync.dma_start(x_all[:, 1, :], x_t[:, 1, :])
            nc.sync.dma_start(x_all[:, 2:4, :], x_t[:, 2:4, :])

            # on-chip constants: ident[p,j] = (j - p == 0), ones, eps (no DMAs
            # -> less HWDGE descriptor serialization at startup).
            iota_d = const.tile([P, P], I16)
            nc.gpsimd.iota(iota_d[:], pattern=[[1, P]], base=0, channel_multiplier=-1)
            ident = const.tile([P, P], BF16)
            nc.vector.tensor_scalar(out=ident[:], in0=iota_d[:], scalar1=0,
                                    scalar2=None, op0=ALU.is_equal)
            ident_f8 = const.tile([P, P], FP8)
            nc.vector.tensor_scalar(out=ident_f8[:], in0=iota_d[:], scalar1=0,
                                    scalar2=None, op0=ALU.is_equal)
            ones_s = const.tile([1, P], FP8)
            nc.vector.memset(ones_s[:], 1.0)
            eps_t = const.tile([P, 1], F32)
            nc.vector.memset(eps_t[:], 1e-5)


            brow_s = const.tile([1, 4 * C], FP8)
            nc.sync.dma_start(brow_s[:], brows.ap())
            wvec_s = const.tile([32, C], BF16)
            nc.sync.dma_start(wvec_s[:], w_vec4.ap())
            b1c_s = const.tile([P, HID // P], F32)
            nc.sync.dma_start(b1c_s[:], b1_col.ap())

            # weight prefetch (Pool queue; overlaps P1)
            wqkv_s = const.tile([P, C // P, 3 * C], FP8, name="wqkv")
            nc.gpsimd.dma_start(wqkv_s[:], w_qkv.ap().rearrange("(ko p) n -> p ko n", p=P))
            wo_s = const.tile([P, C // P, C], FP8, name="wo")
            nc.gpsimd.dma_start(wo_s[:], w_o.ap().rearrange("(ko p) n -> p ko n", p=P))
            for xm in range(4, 16, 2):
                nc.sync.dma_start(x_all[:, xm:xm + 2, :], x_t[:, xm:xm + 2, :])
            # FFN weights are needed only in the tail: load them after x.
            w1_s = const.tile([P, C // P, HID], FP8, name="w1")
            nc.gpsimd.dma_start(w1_s[:], w_1.ap().rearrange("(ko p) n -> p ko n", p=P))
            w2_s = const.tile([P, HID // P, C], FP8, name="w2")
            nc.gpsimd.dma_start(w2_s[:], w_2.ap().rearrange("(ko p) n -> p ko n", p=P))

            x2_all = const.tile([P, NW, C], BF16)
            sx_all = const.tile([P, NW], F32)    # per-window sum(x2) (LN2)
            sx2_all = const.tile([P, NW], F32)   # per-window sum(x2^2)
            zt_sbuf = const.tile([P, C // P, RS], FP8)
            q_sbuf = const.tile([P, NW, C], BF16)
            hT_sbuf = const.tile([P, HID // P, RS], FP8)

            kv_shard = dram.tile([RS, KVB], U8)
            if "ag" not in _SKIP:
                kvt = dram.tile([L, KVB], U8, addr_space="Shared")
            else:
                kvt = dram.tile([L, KVB], U8)

            # Edge-phase loads for the first windows issued BEFORE P1 so they
            # prefetch during P1 (the SP ring is in-order; emitting them after
            # P1's kv stores would delay them to the end of P1).
            edge_tiles = {}
            for w in range(3):
                NT = nts[w]
                to = int(toff[w])
                s12_t = big.tile([P, NTmax, 2, P], BF16, tag="s12", bufs=3)
                nc.sync.dma_start(s12_t[:, 0:NT, :, :],
                                  s12.ap()[:, to:to + NT, :, :])
                rb_t = big.tile([P, NTmax, 12], BF16, tag="rb", bufs=3)
                nc.sync.dma_start(rb_t[:, 0:NT, :],
                                  relbias.ap()[:, to:to + NT, :])
                edge_tiles[w] = (s12_t, rb_t)

            # ---------- LN helper: stats + normalized bf16 z (no affine) ----
            def ln_win(wk, ptp, src, m, copy_eng="v"):
                stats = wk.tile([P, 6], F32, tag="ln_st")
                nc.vector.bn_stats(stats[:], src)
                mv = wk.tile([P, 2], F32, tag="ln_mv")
                nc.vector.bn_aggr(mv[:], stats[:])
                sd = wk.tile([P, 1], F32, tag="ln_sd")
                nc.scalar.activation(sd[:], mv[:, 1:2], AF.Sqrt, bias=eps_t[:], scale=1.0)
                rs_ = wk.tile([P, 1], F32, tag="ln_rs")
                nc.vector.reciprocal(rs_[:], sd[:])
                nmr = wk.tile([P, 1], F32, tag="ln_nmr")
                nc.vector.scalar_tensor_tensor(
                    out=nmr[:], in0=mv[:, 0:1], scalar=-1.0, in1=rs_[:],
                    op0=ALU.mult, op1=ALU.mult)
                zf = wk.tile([P, C], BF16, tag="ln_zf")
                nc.scalar.activation(zf[:], src, AF.Identity, bias=nmr[:], scale=rs_[:])
                tp = ptp.tile([P, C], BF16, tag="tp")
                for c4 in range(C // P):
                    nc.tensor.transpose(tp[:, c4 * P:(c4 + 1) * P],
                                        zf[:, c4 * P:(c4 + 1) * P], ident[:])
                if copy_eng == "a":
                    nc.scalar.activation(
                        zt_sbuf[:, :, m * P:(m + 1) * P],
                        tp[:].rearrange("p (c4 q) -> p c4 q", q=P), AF.Identity)
                elif copy_eng == "g":
                    nc.gpsimd.tensor_copy(
                        zt_sbuf[:, :, m * P:(m + 1) * P],
                        tp[:].rearrange("p (c4 q) -> p c4 q", q=P))
                else:
                    nc.vector.tensor_copy(
                        zt_sbuf[:, :, m * P:(m + 1) * P],
                        tp[:].rearrange("p (c4 q) -> p c4 q", q=P))

            # ============ P1: LN1 + QKV ============
            if "p1" not in _SKIP:
             with tc.tile_pool(name="p1", bufs=4) as wk, \
                 tc.tile_pool(name="p1tp", bufs=4, space="PSUM") as ptp, \
                 tc.tile_pool(name="p1ps", bufs=4, space="PSUM") as pps:
                kv_sh_t = kv_shard[:].rearrange("(m p) n -> p m n", p=P)
                for m in range(NW):
                    # zt copies alternate Act/DVE; nothing from P1 runs on the
                    # Pool queue, so gather(0)'s descriptor-gen is not blocked
                    # behind P1 (Pool is in-order).
                    ln_win(wk, ptp, x_all[:, m, :], m,
                           copy_eng=("a" if m % 2 == 0 else "v"))
                    kvb = wk.tile([P, KVB], U8, tag="kvb")
                    for nb in range(3):
                        ps = pps.tile([P, C], F32, tag="ps")
                        for ko in range(0, C // P, 2):
                            nc.tensor.matmul(
                                ps[:],
                                lhsT=zt_sbuf[:, ko:ko + 2, m * P:(m + 1) * P],
                                rhs=wqkv_s[:, ko:ko + 2, nb * C:(nb + 1) * C],
                                start=(ko == 0), stop=(ko == C // P - 2 and nb != 0),
                                perf_mode=mybir.MatmulPerfMode.DoubleRow)
                        if nb == 0:
                            nc.tensor.matmul(ps[:], lhsT=ones_s[:], rhs=brow_s[0:1, 0:C],
                                             start=False, stop=True)
                            nc.scalar.activation(q_sbuf[:, m, :], ps[:], AF.Identity,
                                                 scale=1.0 / WS)
                        else:
                            # k stored negated (Wk negated host-side) so the edge
                            # phase accumulates qe + (-k) on the PE via ident.
                            # k cast on Act, v on DVE; one combined DMA.
                            if nb == 1:
                                nc.scalar.mul(kvb[:, 0:C].bitcast(FP8), ps[:], 1.0 / WS)
                            else:
                                nc.vector.tensor_scalar_mul(
                                    kvb[:, C:KVB].bitcast(BF16), ps[:], 1.0 / WS)
                    nc.sync.dma_start(kv_sh_t[:, m, :], kvb[:])

            # ============ P2: AllGather ============
            if "ag" not in _SKIP:
                nc.gpsimd.collective_compute(
                    "AllGather", ALU.bypass, replica_groups=[list(range(NCORES))],
                    ins=[kv_shard[:].opt()], outs=[kvt[:].opt()])

            # ============ P3: edge windows + Wo + residual ============
            # `big` lives at top level so s12/idx DMAs and gathers are not
            # WAR-serialized behind P1's SBUF.
            if "edge" not in _SKIP:
             with tc.tile_pool(name="ew", bufs=5) as wk, \
                 tc.tile_pool(name="pqe", bufs=1, space="PSUM") as pqe, \
                 tc.tile_pool(name="ppsV", bufs=2, space="PSUM") as ppsV, \
                 tc.tile_pool(name="ppsA", bufs=2, space="PSUM") as ppsA, \
                 tc.tile_pool(name="ptpc", bufs=1, space="PSUM") as ptpc, \
                 tc.tile_pool(name="p5ps", bufs=1, space="PSUM") as p5ps:
                # tail(w): den -> rden -> anr -> an_ts -> wvec-mm -> attin ->
                # transpose -> Wo-mm -> x2.  Emitted one window late (split in
                # two parts interleaved with window w+1's groups) so its long
                # cross-engine latency chain overlaps the next window's bulk
                # work instead of stalling the in-order engine queues.
                def tail_a(w, psV, psA):
                    den = wk.tile([P, 16], F32, tag="den")
                    nc.vector.tensor_scalar_max(den[:], psA[:, 0:16], 1e-30)
                    rden = wk.tile([P, 16], F32, tag="rden")
                    nc.vector.reciprocal(rden[:], den[:])
                    # fold the (unnormalized) Wvec term into psV: w_vec4 is
                    # head-block-diagonal, so per-(row,head) rden factors
                    # through the sum.
                    anr = wk.tile([P, 32], BF16, tag="anr")
                    nc.scalar.activation(anr[:], psA[:, 16:48], AF.Identity)
                    tpc = ptpc.tile([P, C], BF16, tag="tpc")
                    nc.tensor.transpose(tpc[0:32, 0:P], anr[:], ident[:])
                    an_ts = wk.tile([32, P], BF16, tag="an_ts")
                    nc.scalar.activation(an_ts[:], tpc[0:32, 0:P], AF.Identity)
                    nc.tensor.matmul(psV[:], lhsT=an_ts[:], rhs=wvec_s[:],
                                     start=False, stop=True)
                    attin = wk.tile([P, C], BF16, tag="attin")
                    nc.vector.tensor_mul(
                        attin[:].rearrange("p (h d) -> p h d", h=H),
                        psV[:].rearrange("p (h d) -> p h d", h=H),
                        rden[:].rearrange("p (h j) -> p h j", h=H)[:, :, 0:1]
                            .broadcast_to([P, H, DH]))
                    return attin

                def tail_b(w, attin):
                    tpa = ptpc.tile([P, C], BF16, tag="tpc")
                    for c4 in range(C // P):
                        nc.tensor.transpose(tpa[:, c4 * P:(c4 + 1) * P],
                                            attin[:, c4 * P:(c4 + 1) * P], ident[:])
                    at_sb = wk.tile([P, C // P, P], FP8, tag="at_sb")
                    nc.scalar.activation(
                        at_sb[:], tpa[:].rearrange("p (c4 q) -> p c4 q", q=P),
                        AF.Identity)
                    x2ps = p5ps.tile([P, C], F32, tag="p5")
                    for ko in range(0, C // P, 2):
                        nc.tensor.matmul(x2ps[:], lhsT=at_sb[:, ko:ko + 2, :],
                                         rhs=wo_s[:, ko:ko + 2, :],
                                         start=(ko == 0), stop=False,
                                         perf_mode=mybir.MatmulPerfMode.DoubleRow)
                    nc.tensor.matmul(x2ps[:], lhsT=ones_s[:], rhs=brow_s[0:1, C:2 * C],
                                     start=False, stop=True)
                    nc.vector.scalar_tensor_tensor(
                        out=x2_all[:, w, :], in0=x2ps[:], scalar=1.0 / WS,
                        in1=x_all[:, w, :], op0=ALU.mult, op1=ALU.add)
                    # LN2 stats via the Act accumulator (Act has slack in the
                    # edge phase; keeps the bn_stats chain out of the tail).
                    trash = wk.tile([P, C], BF16, tag="trash")
                    nc.scalar.activation(trash[:], x2_all[:, w, :], AF.Square,
                                         accum_out=sx2_all[:, w:w + 1])
                    nc.scalar.activation(trash[:], x2_all[:, w, :], AF.Identity,
                                         accum_out=sx_all[:, w:w + 1])

                prev = None       # (w, psV, psA) of the previous window
                prev_attin = None  # (w, attin) pending tail_b
                for w in range(NW):
                    NT = nts[w]
                    TW = NT * P
                    eo = sum(n * P // 16 for n in nts[:w])
                    to = int(toff[w])
                    if w in edge_tiles:
                        s12_t, rb_t = edge_tiles.pop(w)
                    else:
                        s12_t = big.tile([P, NTmax, 2, P], BF16, tag="s12", bufs=3)
                        nc.sync.dma_start(s12_t[:, 0:2, :, :],
                                          s12.ap()[:, to:to + 2, :, :])
                        nc.sync.dma_start(s12_t[:, 2:NT, :, :],
                                          s12.ap()[:, to + 2:to + NT, :, :])
                        rb_t = big.tile([P, NTmax, 12], BF16, tag="rb", bufs=3)
                        nc.sync.dma_start(rb_t[:, 0:NT, :],
                                          relbias.ap()[:, to:to + NT, :])
                    idx_t = big.tile([P, NTmax * P // 16], I16, tag="idx", bufs=3)
                    nc.sync.dma_start(idx_t[:, 0:TW // 16],
                                      eidx.ap()[:, eo:eo + TW // 16])
                    kv_g = big.tile([P, NTmax, KVB], U8, tag="kv", bufs=3)
                    # split each gather into quarter-gathers alternating the
                    # two SWDGE queues: the first chunk lands earlier (qe for
                    # the first tiles starts sooner) and rings stay pipelined.
                    QC = 1  # tiles per gather chunk
                    for ci, c0 in enumerate(range(0, NT, QC)):
                        cb = min(QC, NT - c0)
                        nc.gpsimd.dma_gather(
                            out_ap=kv_g[:, c0:c0 + cb, :], in_ap=kvt[:],
                            idxs_ap=idx_t[:, c0 * P // 16:(c0 + cb) * P // 16],
                            num_idxs=cb * P, num_idxs_reg=cb * P, elem_size=KVB,
                            single_packet=False, queue_num=ci % 2)

                    psV = ppsV.tile([P, 512], F32, tag="psV")
                    psA = ppsA.tile([P, 48], F32, tag="psA")
                    ngrp = (NT + GS - 1) // GS
                    for gi, t0 in enumerate(range(0, NT, GS)):
                        tb = min(GS, NT - t0)
                        qe = pqe.tile([P, GS, C], F32, tag="qe")
                        for d_ in range(tb):
                            nc.tensor.matmul(qe[:, d_, :],
                                             lhsT=s12_t[:, t0 + d_, 1, :],
                                             rhs=q_sbuf[:, w, :], start=True, stop=False)
                            nc.tensor.matmul(qe[:, d_, :], lhsT=ident_f8[:],
                                             rhs=kv_g[:, t0 + d_, 0:C].bitcast(FP8),
                                             start=False, stop=True)
                        if gi == 1 and prev is not None:
                            if prev_attin is not None:
                                tail_b(*prev_attin)
                            prev_attin = (prev[0], tail_a(*prev))
                            prev = None
                        dsq = wk.tile([P, GS, C], BF16, tag="dsq")
                        nc.scalar.activation(dsq[:, 0:tb, :], qe[:, 0:tb, :], AF.Square)
                        # staged-halving reduce: two bf16 2x-mode adds, then a
                        # short TensorReduce (TensorReduce has no fast mode).
                        d4 = dsq[:, 0:tb, :].rearrange("p t (h j d) -> p t h j d", h=H, j=2)
                        h1 = wk.tile([P, GS, H, DH // 2], BF16, tag="h1")
                        nc.vector.tensor_add(h1[:, 0:tb, :, :], d4[:, :, :, 0, :], d4[:, :, :, 1, :])
                        h14 = h1[:, 0:tb, :, :].rearrange("p t h (j d) -> p t h j d", j=2)
                        h2 = wk.tile([P, GS, H, DH // 4], BF16, tag="h2")
                        nc.vector.tensor_add(h2[:, 0:tb, :, :], h14[:, :, :, 0, :], h14[:, :, :, 1, :])
                        s8 = wk.tile([P, GS, H], F32, tag="s8")
                        nc.vector.reduce_sum(
                            s8[:, 0:tb, :], h2[:, 0:tb, :, :], axis=AX.X)
                        sc = wk.tile([P, GS, H], F32, tag="sc")
                        nc.vector.scalar_tensor_tensor(
                            out=sc[:, 0:tb, :], in0=s8[:, 0:tb, :], scalar=-inv_s,
                            in1=rb_t[:, t0:t0 + tb, 4:12], op0=ALU.mult, op1=ALU.add)
                        eaux = wk.tile([P, GS, 48], BF16, tag="eaux")
                        nc.scalar.activation(
                            eaux[:, 0:tb, 0:16].rearrange("p t (h j) -> p t h j", h=H),
                            sc[:, 0:tb, :].unsqueeze(3).broadcast_to([P, tb, H, 2]),
                            AF.Exp)
                        pev = wk.tile([P, GS, C], BF16, tag="pev")
                        for d_ in range(tb):
                            nc.vector.tensor_mul(
                                pev[:, d_, :].rearrange("p (h a j) -> p h a j", h=H, j=2),
                                eaux[:, d_, 0:16].rearrange("p (h j) -> p h j", h=H)
                                    .unsqueeze(2).broadcast_to([P, H, DH // 2, 2]),
                                kv_g[:, t0 + d_, C:KVB].bitcast(BF16)
                                    .rearrange("p (h a j) -> p h a j", h=H, j=2))
                        nc.vector.tensor_mul(
                            eaux[:, 0:tb, 16:48].rearrange("p t (h a) -> p t h a", h=H),
                            eaux[:, 0:tb, 0:16].rearrange("p t (h j) -> p t h j", h=H)[:, :, :, 0:1]
                                .broadcast_to([P, tb, H, 4]),
                            rb_t[:, t0:t0 + tb, 0:4].unsqueeze(2)
                                .broadcast_to([P, tb, H, 4]))
                        for d_ in range(tb):
                            t = t0 + d_
                            nc.tensor.matmul(psV[:], lhsT=s12_t[:, t, 0, :],
                                             rhs=pev[:, d_, :],
                                             start=(t == 0), stop=False)
                            nc.tensor.matmul(psA[:], lhsT=s12_t[:, t, 0, :],
                                             rhs=eaux[:, d_, :],
                                             start=(t == 0), stop=(t == NT - 1))

                    prev = (w, psV, psA)
                if prev is not None:
                    if prev_attin is not None:
                        tail_b(*prev_attin)
                    prev_attin = (prev[0], tail_a(*prev))
                if prev_attin is not None:
                    tail_b(*prev_attin)

            # ============ P4: LN2 + FFN ============
            if "p4" not in _SKIP:
             with tc.tile_pool(name="p4", bufs=4) as wk, \
                 tc.tile_pool(name="p4tp", bufs=4, space="PSUM") as ptp, \
                 tc.tile_pool(name="f1ps", bufs=2, space="PSUM") as pps1, \
                 tc.tile_pool(name="f2ps", bufs=2, space="PSUM") as pps2:
                # mean/var from the Act-accumulated sums: mean = sx/C,
                # var = sx2/C - mean^2; one batched Sqrt (one table switch).
                mean_all = wk.tile([P, NW], F32, tag="mean_all")
                nc.vector.tensor_scalar_mul(mean_all[:], sx_all[:], 1.0 / C)
                msq = wk.tile([P, NW], F32, tag="msq")
                nc.vector.tensor_mul(msq[:], mean_all[:], mean_all[:])
                var_all = wk.tile([P, NW], F32, tag="var_all")
                nc.vector.scalar_tensor_tensor(
                    out=var_all[:], in0=sx2_all[:], scalar=1.0 / C,
                    in1=msq[:], op0=ALU.mult, op1=ALU.subtract)
                sd_all = wk.tile([P, NW], F32, tag="sd_all")
                nc.scalar.activation(sd_all[:], var_all[:], AF.Sqrt,
                                     bias=eps_t[:], scale=1.0)
                rs_all = wk.tile([P, NW], F32, tag="rs_all")
                nc.vector.reciprocal(rs_all[:], sd_all[:])
                nmr_all = wk.tile([P, NW], F32, tag="nmr_all")
                nc.vector.scalar_tensor_tensor(
                    out=nmr_all[:], in0=mean_all[:], scalar=-1.0,
                    in1=rs_all[:], op0=ALU.mult, op1=ALU.mult)
                for m in range(NW):
                    # zf on DVE (2x mode, per-partition AP scalars): frees Act
                    # for the gelus.
                    zf = wk.tile([P, C], BF16, tag="ln_zf")
                    nc.vector.tensor_scalar(
                        out=zf[:], in0=x2_all[:, m, :],
                        scalar1=rs_all[:, m:m + 1], scalar2=nmr_all[:, m:m + 1],
                        op0=ALU.mult, op1=ALU.add)
                    tp = ptp.tile([P, C], BF16, tag="tp")
                    for c4 in range(C // P):
                        nc.tensor.transpose(tp[:, c4 * P:(c4 + 1) * P],
                                            zf[:, c4 * P:(c4 + 1) * P], ident[:])
                    if m % 2 == 0:
                        nc.scalar.activation(
                            zt_sbuf[:, :, m * P:(m + 1) * P],
                            tp[:].rearrange("p (c4 q) -> p c4 q", q=P), AF.Identity)
                    else:
                        nc.vector.tensor_copy(
                            zt_sbuf[:, :, m * P:(m + 1) * P],
                            tp[:].rearrange("p (c4 q) -> p c4 q", q=P))
                for rc in range(RS // 512):
                    for ht in range(HID // P):
                        ps = pps1.tile([P, 512], F32, tag="ps1")
                        for ko in range(0, C // P, 2):
                            nc.tensor.matmul(
                                ps[:], lhsT=w1_s[:, ko:ko + 2, ht * P:(ht + 1) * P],
                                rhs=zt_sbuf[:, ko:ko + 2, rc * 512:(rc + 1) * 512],
                                start=(ko == 0), stop=(ko == C // P - 2),
                                perf_mode=mybir.MatmulPerfMode.DoubleRow)
                        nc.scalar.activation(
                            hT_sbuf[:, ht, rc * 512:(rc + 1) * 512], ps[:],
                            AF.Gelu_apprx_tanh, bias=b1c_s[:, ht:ht + 1], scale=1.0 / WS)
                    for m in range(rc * 4, rc * 4 + 4):
                        ps = pps2.tile([P, C], F32, tag="ps2")
                        for ht in range(0, HID // P, 2):
                            nc.tensor.matmul(ps[:], lhsT=hT_sbuf[:, ht:ht + 2, m * P:(m + 1) * P],
                                             rhs=w2_s[:, ht:ht + 2, :],
                                             start=(ht == 0), stop=False,
                                             perf_mode=mybir.MatmulPerfMode.DoubleRow)
                        nc.tensor.matmul(ps[:], lhsT=ones_s[:], rhs=brow_s[0:1, 2 * C:3 * C],
                                         start=False, stop=True)
                        if m % 2 == 0:
                            yt = wk.tile([P, 2, C], F32, tag="y", bufs=2)
                        nc.vector.scalar_tensor_tensor(
                            out=yt[:, m % 2, :], in0=ps[:], scalar=1.0 / WS,
                            in1=x2_all[:, m, :], op0=ALU.mult, op1=ALU.add)
                        if m % 2 == 1:
                            nc.sync.dma_start(y_t[:, m - 1:m + 1, :], yt[:])

    nc.compile()
    return nc


def _prep(inputs):
    row = np.asarray(inputs["row_index"]).astype(np.int64).ravel()
    col = np.asarray(inputs["col_index"]).astype(np.int64).ravel()
    tcol = np.asarray(inputs["to_col_index"]).astype(np.int64).ravel()
    bias = np.asarray(inputs["pos_att_bias"], dtype=np.float32)
    dist = np.asarray(inputs["dist"], dtype=np.float32).ravel()
    pos = np.asarray(inputs["pos"], dtype=np.float32)
    cpos = np.asarray(inputs["col_pos"], dtype=np.float32)

    # ---- balance rows into 128 bins (8 cores x 16 windows, 128 rows each) ---
    import heapq
    NB = NCORES * NW
    cnt = np.bincount(row, minlength=L)
    order_r = np.argsort(-cnt, kind="stable")
    heap = [(0, 0, b) for b in range(NB)]
    heapq.heapify(heap)
    bin_rows = [[] for _ in range(NB)]
    bin_sum = np.zeros(NB, np.int64)
    for r in order_r:
        popped = []
        while True:
            s, n, b = heapq.heappop(heap)
            if n < P:
                break
            popped.append((s, n, b))
        for x_ in popped:
            heapq.heappush(heap, x_)
        bin_rows[b].append(int(r))
        bin_sum[b] = s + cnt[r]
        heapq.heappush(heap, (int(bin_sum[b]), n + 1, b))

    # snake-assign bins to cores by edge-count rank; windows sorted descending
    # within each core so window j's count is similar across cores.
    rk = np.argsort(-bin_sum)
    core_bins = [[] for _ in range(NCORES)]
    for i, b in enumerate(rk):
        core_bins[i % NCORES].append(int(b))
    for c in range(NCORES):
        core_bins[c].sort(key=lambda b: -int(bin_sum[b]))
    counts = np.array([[bin_sum[b] for b in core_bins[c]] for c in range(NCORES)])
    nts = tuple(int(x) for x in np.ceil(counts.max(axis=0) / P).astype(int))

    # global row permutation: new row (c*RS + w*P + slot) = old row
    perm = np.empty(L, np.int64)
    for c in range(NCORES):
        for w in range(NW):
            b = core_bins[c][w]
            perm[c * RS + w * P:c * RS + (w + 1) * P] = bin_rows[b]
    inv_perm = np.empty(L, np.int64)
    inv_perm[perm] = np.arange(L)

    new_row = inv_perm[row]   # position of each edge's target row
    new_col = inv_perm[col]   # position of each edge's source col in permuted kvt

    TOT = sum(nts)
    E16 = sum(n * P // 16 for n in nts)
    toff = np.concatenate([[0], np.cumsum(nts)]).astype(int)
    eoff = np.concatenate([[0], np.cumsum([n * P // 16 for n in nts])]).astype(int)

    eidx_h = np.zeros((NCORES, P, E16), np.int16)
    rb_h = np.zeros((NCORES, P, TOT, 12), np.float32)
    rb_h[:, :, :, 4:12] = -1e4
    s12_h = np.zeros((NCORES, P, TOT, 2, P), np.float32)

    gw_all = new_row // P  # global window id (0..127) per edge
    order_e = np.argsort(gw_all, kind="stable")
    gw_s = gw_all[order_e]
    starts = np.searchsorted(gw_s, np.arange(NB + 1))
    for gw in range(NB):
        c, w = divmod(gw, NW)
        sl = order_e[starts[gw]:starts[gw + 1]]
        n = len(sl)
        if n == 0:
            continue
        TWw = nts[w] * P
        assert n <= TWw, (n, TWw)
        erows = (new_row[sl] - gw * P).astype(np.int64)
        ecols = new_col[sl]
        j = np.arange(n)
        wrap = np.zeros((16, TWw // 16), np.int16)
        wrap[j % 16, j // 16] = ecols.astype(np.int16)
        eidx_h[c, :, eoff[w]:eoff[w + 1]] = np.tile(wrap, (8, 1))
        t_of = toff[w] + j // P
        e_of = j % P
        rb_h[c, e_of, t_of, 0:3] = (cpos[tcol[sl]] - pos[row[sl]]) / dist[sl][:, None]
        rb_h[c, e_of, t_of, 3] = 1.0
        rb_h[c, e_of, t_of, 4:12] = bias[sl]
        s12_h[c, e_of, t_of, 0, erows] = 1.0   # s1: edge -> row scatter
        s12_h[c, erows, t_of, 1, e_of] = 1.0   # s2: row -> edge expand
    import ml_dtypes
    bf = ml_dtypes.bfloat16
    return nts, perm, eidx_h, rb_h.astype(bf), s12_h.astype(bf)


def kernel(**inputs):
    import ml_dtypes
    bf = ml_dtypes.bfloat16
    x = np.asarray(inputs["x"], dtype=np.float32)
    nts, perm, eidx_h, rb_h, s12_h = _prep(inputs)
    if nts not in _cache:
        _cache[nts] = _build(nts)
    nc = _cache[nts]

    f32 = lambda k: np.asarray(inputs[k], np.float32)
    g1, b1l = f32("ln1_g"), f32("ln1_b")
    g2, b2l = f32("ln2_g"), f32("ln2_b")
    Wq, Wk, Wv, Wo = f32("Wq"), f32("Wk"), f32("Wv"), f32("Wo")
    # Fold LN affine into the following matmuls; fold bk into bq (only the
    # difference q-k matters) and bv into bo (sum_e alpha = 1 per head).
    Wq_, Wk_, Wv_ = g1[:, None] * Wq, g1[:, None] * Wk, g1[:, None] * Wv
    bq_ = (b1l @ Wq + f32("bq")) - (b1l @ Wk + f32("bk"))
    bo_ = (b1l @ Wv + f32("bv")) @ Wo + f32("bo")
    W1_ = g2[:, None] * f32("W1")
    b1_ = b2l @ f32("W1") + f32("b1")
    import ml_dtypes as _md
    f8 = _md.float8_e4m3
    WS = 64.0
    # Wk negated: the kernel stores k pre-negated for the PE qe-k accumulate.
    w_qkv = (np.concatenate([Wq_, -Wk_, Wv_], axis=1) * WS).astype(f8)

    wv4 = np.concatenate([f32("Wvec"), f32("bvec")[None, :]], axis=0)
    w_vec4 = np.zeros((32, C), np.float32)
    for h in range(H):
        w_vec4[4 * h:4 * h + 4, h * DH:(h + 1) * DH] = wv4[:, h * DH:(h + 1) * DH]

    brows = np.zeros((1, 4 * C), np.float32)
    brows[0, 0:C] = bq_
    brows[0, C:2 * C] = bo_
    brows[0, 2 * C:3 * C] = f32("b2")
    b1_col = np.ascontiguousarray(b1_.reshape(HID // P, P).T)

    xp = x[perm]
    in_maps = []
    for c in range(NCORES):
        in_maps.append(dict(
            x_in=np.ascontiguousarray(xp[c * RS:(c + 1) * RS]),
            w_qkv=w_qkv, w_o=(Wo * WS).astype(f8),
            w_1=(W1_ * WS).astype(f8), w_2=(f32("W2") * WS).astype(f8),
            w_vec4=w_vec4.astype(bf), b1_col=b1_col,
            brows=(brows * WS).astype(f8),
            eidx=eidx_h[c], relbias=rb_h[c], s12=s12_h[c],
        ))
    _last["nc"] = nc
    _last["in_maps"] = in_maps
    res = run_bass_kernel_spmd(nc, in_maps, list(range(NCORES)))
    global _last_res
    _last_res = res
    yp = np.concatenate([res.results[c]["y_out"] for c in range(NCORES)], axis=0)
    y = np.empty_like(yp)
    y[perm] = yp
    return np.asarray(y, np.float32)


_last = {}
_last_res = None


# revision 83
# speedup vs baseline: 1.2877x; 1.0120x over previous
"""Trainium2 Bass kernel for nn_EncoderLayer_88476326298146 (sparse graph attention).

Row-sharded across 8 NeuronCores with host-side load balancing: all L rows are
LPT-packed into 128 bins (8 cores x 16 windows, exactly 128 rows each) so edge
counts per window are near-uniform; per-window tile counts (nts) are baked into
the build. k/v (k fp8 negated via host-negated Wk, v bf16; 1.5KB/row) are replicated via
AllGather; per-edge col features come from per-tile dma_gather chunks
alternating the two SWDGE queues (early chunks land sooner, and one full-window
gather would fill a whole 1024-descriptor ring).

- LN affine folded into following weights host-side; biases via rank-1 ones-row
  matmuls on the PE.
- diff = q_row - k_col on the PE (s2^T@q then accumulate ident@(-k)); square on
  Act from PSUM; per-head reduce = two bf16 2x-mode halving adds + short reduce.
- exp emitted as bf16 PAIRS (eaux[...,0:16]) and shared by the alpha*v multiply
  (DVE 2x) and the aux (den/rel) matmul.
- segment softmax with m=0; segment sums via host-built one-hot matmuls.
- FFN1 produced transposed with gelu+bias fused on Act; FFN2 consumes h^T as
  lhsT. LN2 uses one batched Sqrt so the act table switches only once.
- DMA batching: s1+s2 in one tensor, rel+bias in one bf16 tensor, k+v in one
  store per window; ident/ones/eps generated on-chip.
"""
import os
import numpy as np

import concourse.bass as bass
import concourse.bacc as bacc
import concourse.mybir as mybir
import concourse.tile as tile
from concourse.bass_utils import run_bass_kernel_spmd
from concourse.library_config import mlp as mlp_lib

L, E, SP, C, H, DH, HID = 16384, 131072, 20000, 512, 8, 64, 1024
NCORES = 8
RS = L // NCORES
NW = RS // 128
P = 128
F32 = mybir.dt.float32
BF16 = mybir.dt.bfloat16
I16 = mybir.dt.int16
FP8 = mybir.dt.float8e4
U8 = mybir.dt.uint8
KVB = 3 * C  # kv row bytes: k fp8 (C) + v bf16 (2C)
WS = 64.0  # weight pre-scale (fp8 subnormal avoidance); descaled in Act casts
AF = mybir.ActivationFunctionType
ALU = mybir.AluOpType
AX = mybir.AxisListType

_cache = {}
_SKIP = set(os.environ.get("KSKIP", "").split(","))


def _build(nts):
    if isinstance(nts, int):
        nts = (nts // P,) * NW
    nts = tuple(int(n) for n in nts)
    assert len(nts) == NW
    NTmax = max(nts)
    TOT = sum(nts)             # total tiles across windows
    E16 = sum(n * P // 16 for n in nts)  # eidx columns
    toff = np.concatenate([[0], np.cumsum(nts)]).astype(int)
    GS = 2  # tiles per score group (PSUM: GS banks for qe)
    inv_s = 1.0 / float(np.sqrt(DH))
    nc = bacc.Bacc("TRN2", target_bir_lowering=False, debug=False, num_devices=NCORES,
                   num_swdge_queues=2)

    x_in = nc.dram_tensor("x_in", [RS, C], F32, kind="ExternalInput")
    w_qkv = nc.dram_tensor("w_qkv", [C, 3 * C], FP8, kind="ExternalInput")
    w_o = nc.dram_tensor("w_o", [C, C], FP8, kind="ExternalInput")
    w_1 = nc.dram_tensor("w_1", [C, HID], FP8, kind="ExternalInput")
    w_2 = nc.dram_tensor("w_2", [HID, C], FP8, kind="ExternalInput")
    w_vec4 = nc.dram_tensor("w_vec4", [32, C], BF16, kind="ExternalInput")
    b1_col = nc.dram_tensor("b1_col", [P, HID // P], F32, kind="ExternalInput")
    brows = nc.dram_tensor("brows", [1, 4 * C], FP8, kind="ExternalInput")
    eidx = nc.dram_tensor("eidx", [P, E16], I16, kind="ExternalInput")
    relbias = nc.dram_tensor("relbias", [P, TOT, 12], BF16, kind="ExternalInput")
    s12 = nc.dram_tensor("s12", [P, TOT, 2, P], BF16, kind="ExternalInput")
    y_out = nc.dram_tensor("y_out", [RS, C], F32, kind="ExternalOutput")

    x_t = x_in.ap().rearrange("(m p) n -> p m n", p=P)
    y_t = y_out.ap().rearrange("(m p) n -> p m n", p=P)

    with tile.TileContext(nc) as tc:
        with tc.tile_pool(name="dram", bufs=1, space="DRAM") as dram, \
             tc.tile_pool(name="const", bufs=1) as const, \
             tc.tile_pool(name="big", bufs=2) as big:
            nc.gpsimd.load_library(mlp_lib)

            # x loads first: LN(0) is the startup critical path.
            x_all = const.tile([P, NW, C], F32)
            nc.sync.dma_start(x_all[:, 0, :], x_t[:, 0, :])
            nc.sync.dma_start(x_all[:, 1, :], x_t[:, 1, :])
            nc.sync.dma_start(x_all[:, 2:4, :], x_t[:, 2:4, :])

            # on-chip constants: ident[p,j] = (j - p == 0), ones, eps (no DMAs
            # -> less HWDGE descriptor serialization at startup).
            iota_d = const.tile([P, P], I16)
            nc.gpsimd.iota(iota_d[:], pattern=[[1, P]], base=0, channel_multiplier=-1)
            ident = const.tile([P, P], BF16)
            nc.vector.tensor_scalar(out=ident[:], in0=iota_d[:], scalar1=0,
                                    scalar2=None, op0=ALU.is_equal)
            ident_f8 = const.tile([P, P], FP8)
            nc.vector.tensor_scalar(out=ident_f8[:], in0=iota_d[:], scalar1=0,
                                    scalar2=None, op0=ALU.is_equal)
            ones_s = const.tile([1, P], FP8)
            nc.vector.memset(ones_s[:], 1.0)
            eps_t = const.tile([P, 1], F32)
            nc.vector.memset(eps_t[:], 1e-5)


            brow_s = const.tile([1, 4 * C], FP8)
            nc.sync.dma_start(brow_s[:], brows.ap())
            wvec_s = const.tile([32, C], BF16)
            nc.sync.dma_start(wvec_s[:], w_vec4.ap())
            b1c_s = const.tile([P, HID // P], F32)
            nc.sync.dma_start(b1c_s[:], b1_col.ap())

            # weight prefetch (Pool queue; overlaps P1)
            wqkv_s = const.tile([P, C // P, 3 * C], FP8, name="wqkv")
            nc.gpsimd.dma_start(wqkv_s[:], w_qkv.ap().rearrange("(ko p) n -> p ko n", p=P))
            wo_s = const.tile([P, C // P, C], FP8, name="wo")
            nc.gpsimd.dma_start(wo_s[:], w_o.ap().rearrange("(ko p) n -> p ko n", p=P))
            for xm in range(4, 16, 2):
                nc.sync.dma_start(x_all[:, xm:xm + 2, :], x_t[:, xm:xm + 2, :])
            # FFN weights are needed only in the tail: load them after x.
            w1_s = const.tile([P, C // P, HID], FP8, name="w1")
            nc.gpsimd.dma_start(w1_s[:], w_1.ap().rearrange("(ko p) n -> p ko n", p=P))
            w2_s = const.tile([P, HID // P, C], FP8, name="w2")
            nc.gpsimd.dma_start(w2_s[:], w_2.ap().rearrange("(ko p) n -> p ko n", p=P))

            x2_all = const.tile([P, NW, C], BF16)
            sx_all = const.tile([P, NW], F32)    # per-window sum(x2) (LN2)
            sx2_all = const.tile([P, NW], F32)   # per-window sum(x2^2)
            zt_sbuf = const.tile([P, C // P, RS], FP8)
            q_sbuf = const.tile([P, NW, C], BF16)
            hT_sbuf = const.tile([P, HID // P, RS], FP8)

            kv_shard = dram.tile([RS, KVB], U8)
            if "ag" not in _SKIP:
                kvt = dram.tile([L, KVB], U8, addr_space="Shared")
            else:
                kvt = dram.tile([L, KVB], U8)

            # Edge-phase loads for the first windows issued BEFORE P1 so they
            # prefetch during P1 (the SP ring is in-order; emitting them after
            # P1's kv stores would delay them to the end of P1).
            edge_tiles = {}
            for w in range(3):
                NT = nts[w]
                to = int(toff[w])
                s12_t = big.tile([P, NTmax, 2, P], BF16, tag="s12", bufs=3)
                nc.sync.dma_start(s12_t[:, 0:NT, :, :],
                                  s12.ap()[:, to:to + NT, :, :])
                rb_t = big.tile([P, NTmax, 12], BF16, tag="rb", bufs=3)
                nc.sync.dma_start(rb_t[:, 0:NT, :],
                                  relbias.ap()[:, to:to + NT, :])
                edge_tiles[w] = (s12_t, rb_t)

            # ---------- LN helper: stats + normalized bf16 z (no affine) ----
            def ln_win(wk, ptp, src, m, copy_eng="v"):
                stats = wk.tile([P, 6], F32, tag="ln_st")
                nc.vector.bn_stats(stats[:], src)
                mv = wk.tile([P, 2], F32, tag="ln_mv")
                nc.vector.bn_aggr(mv[:], stats[:])
                sd = wk.tile([P, 1], F32, tag="ln_sd")
                nc.scalar.activation(sd[:], mv[:, 1:2], AF.Sqrt, bias=eps_t[:], scale=1.0)
                rs_ = wk.tile([P, 1], F32, tag="ln_rs")
                nc.vector.reciprocal(rs_[:], sd[:])
                nmr = wk.tile([P, 1], F32, tag="ln_nmr")
                nc.vector.scalar_tensor_tensor(
                    out=nmr[:], in0=mv[:, 0:1], scalar=-1.0, in1=rs_[:],
                    op0=ALU.mult, op1=ALU.mult)
                zf = wk.tile([P, C], BF16, tag="ln_zf")
                nc.scalar.activation(zf[:], src, AF.Identity, bias=nmr[:], scale=rs_[:])
                tp = ptp.tile([P, C], BF16, tag="tp")
                for c4 in range(C // P):
                    nc.tensor.transpose(tp[:, c4 * P:(c4 + 1) * P],
                                        zf[:, c4 * P:(c4 + 1) * P], ident[:])
                if copy_eng == "a":
                    nc.scalar.activation(
                        zt_sbuf[:, :, m * P:(m + 1) * P],
                        tp[:].rearrange("p (c4 q) -> p c4 q", q=P), AF.Identity)
                elif copy_eng == "g":
                    nc.gpsimd.tensor_copy(
                        zt_sbuf[:, :, m * P:(m + 1) * P],
                        tp[:].rearrange("p (c4 q) -> p c4 q", q=P))
                else:
                    nc.vector.tensor_copy(
                        zt_sbuf[:, :, m * P:(m + 1) * P],
                        tp[:].rearrange("p (c4 q) -> p c4 q", q=P))

            # ============ P1: LN1 + QKV ============
            if "p1" not in _SKIP:
             with tc.tile_pool(name="p1", bufs=4) as wk, \
                 tc.tile_pool(name="p1tp", bufs=4, space="PSUM") as ptp, \
                 tc.tile_pool(name="p1ps", bufs=4, space="PSUM") as pps:
                kv_sh_t = kv_shard[:].rearrange("(m p) n -> p m n", p=P)
                def qkv_part(m):
                    kvb = wk.tile([P, KVB], U8, tag="kvb")
                    for nb in range(3):
                        ps = pps.tile([P, C], F32, tag="ps")
                        for ko in range(0, C // P, 2):
                            nc.tensor.matmul(
                                ps[:],
                                lhsT=zt_sbuf[:, ko:ko + 2, m * P:(m + 1) * P],
                                rhs=wqkv_s[:, ko:ko + 2, nb * C:(nb + 1) * C],
                                start=(ko == 0), stop=(ko == C // P - 2 and nb != 0),
                                perf_mode=mybir.MatmulPerfMode.DoubleRow)
                        if nb == 0:
                            nc.tensor.matmul(ps[:], lhsT=ones_s[:], rhs=brow_s[0:1, 0:C],
                                             start=False, stop=True)
                            nc.scalar.activation(q_sbuf[:, m, :], ps[:], AF.Identity,
                                                 scale=1.0 / WS)
                        else:
                            # k stored negated (Wk negated host-side) so the edge
                            # phase accumulates qe + (-k) on the PE via ident.
                            # k cast on Act, v on DVE; one combined DMA.
                            if nb == 1:
                                nc.scalar.mul(kvb[:, 0:C].bitcast(FP8), ps[:], 1.0 / WS)
                            else:
                                nc.vector.tensor_scalar_mul(
                                    kvb[:, C:KVB].bitcast(BF16), ps[:], 1.0 / WS)
                    nc.sync.dma_start(kv_sh_t[:, m, :], kvb[:])

                # software-pipelined: window m's QKV emitted after window
                # m+1's LN so the LN chain overlaps the previous QKV.
                for m in range(NW):
                    # zt copies alternate Act/DVE; nothing from P1 runs on the
                    # Pool queue, so gather(0)'s descriptor-gen is not blocked
                    # behind P1 (Pool is in-order).
                    ln_win(wk, ptp, x_all[:, m, :], m,
                           copy_eng=("a" if m % 2 == 0 else "v"))
                    if m > 0:
                        qkv_part(m - 1)
                qkv_part(NW - 1)

            # ============ P2: AllGather ============
            if "ag" not in _SKIP:
                nc.gpsimd.collective_compute(
                    "AllGather", ALU.bypass, replica_groups=[list(range(NCORES))],
                    ins=[kv_shard[:].opt()], outs=[kvt[:].opt()])

            # ============ P3: edge windows + Wo + residual ============
            # `big` lives at top level so s12/idx DMAs and gathers are not
            # WAR-serialized behind P1's SBUF.
            if "edge" not in _SKIP:
             with tc.tile_pool(name="ew", bufs=5) as wk, \
                 tc.tile_pool(name="pqe", bufs=1, space="PSUM") as pqe, \
                 tc.tile_pool(name="ppsV", bufs=2, space="PSUM") as ppsV, \
                 tc.tile_pool(name="ppsA", bufs=2, space="PSUM") as ppsA, \
                 tc.tile_pool(name="ptpc", bufs=1, space="PSUM") as ptpc, \
                 tc.tile_pool(name="p5ps", bufs=1, space="PSUM") as p5ps:
                # tail(w): den -> rden -> anr -> an_ts -> wvec-mm -> attin ->
                # transpose -> Wo-mm -> x2.  Emitted one window late (split in
                # two parts interleaved with window w+1's groups) so its long
                # cross-engine latency chain overlaps the next window's bulk
                # work instead of stalling the in-order engine queues.
                def tail_a(w, psV, psA):
                    den = wk.tile([P, 16], F32, tag="den")
                    nc.vector.tensor_scalar_max(den[:], psA[:, 0:16], 1e-30)
                    rden = wk.tile([P, 16], F32, tag="rden")
                    nc.vector.reciprocal(rden[:], den[:])
                    # fold the (unnormalized) Wvec term into psV: w_vec4 is
                    # head-block-diagonal, so per-(row,head) rden factors
                    # through the sum.
                    anr = wk.tile([P, 32], BF16, tag="anr")
                    nc.scalar.activation(anr[:], psA[:, 16:48], AF.Identity)
                    tpc = ptpc.tile([P, C], BF16, tag="tpc")
                    nc.tensor.transpose(tpc[0:32, 0:P], anr[:], ident[:])
                    an_ts = wk.tile([32, P], BF16, tag="an_ts")
                    nc.scalar.activation(an_ts[:], tpc[0:32, 0:P], AF.Identity)
                    nc.tensor.matmul(psV[:], lhsT=an_ts[:], rhs=wvec_s[:],
                                     start=False, stop=True)
                    attin = wk.tile([P, C], BF16, tag="attin")
                    nc.vector.tensor_mul(
                        attin[:].rearrange("p (h d) -> p h d", h=H),
                        psV[:].rearrange("p (h d) -> p h d", h=H),
                        rden[:].rearrange("p (h j) -> p h j", h=H)[:, :, 0:1]
                            .broadcast_to([P, H, DH]))
                    return attin

                def tail_b(w, attin):
                    tpa = ptpc.tile([P, C], BF16, tag="tpc")
                    for c4 in range(C // P):
                        nc.tensor.transpose(tpa[:, c4 * P:(c4 + 1) * P],
                                            attin[:, c4 * P:(c4 + 1) * P], ident[:])
                    at_sb = wk.tile([P, C // P, P], FP8, tag="at_sb")
                    nc.scalar.activation(
                        at_sb[:], tpa[:].rearrange("p (c4 q) -> p c4 q", q=P),
                        AF.Identity)
                    x2ps = p5ps.tile([P, C], F32, tag="p5")
                    for ko in range(0, C // P, 2):
                        nc.tensor.matmul(x2ps[:], lhsT=at_sb[:, ko:ko + 2, :],
                                         rhs=wo_s[:, ko:ko + 2, :],
                                         start=(ko == 0), stop=False,
                                         perf_mode=mybir.MatmulPerfMode.DoubleRow)
                    nc.tensor.matmul(x2ps[:], lhsT=ones_s[:], rhs=brow_s[0:1, C:2 * C],
                                     start=False, stop=True)
                    nc.vector.scalar_tensor_tensor(
                        out=x2_all[:, w, :], in0=x2ps[:], scalar=1.0 / WS,
                        in1=x_all[:, w, :], op0=ALU.mult, op1=ALU.add)
                    # LN2 stats via the Act accumulator (Act has slack in the
                    # edge phase; keeps the bn_stats chain out of the tail).
                    trash = wk.tile([P, C], BF16, tag="trash")
                    nc.scalar.activation(trash[:], x2_all[:, w, :], AF.Square,
                                         accum_out=sx2_all[:, w:w + 1])
                    nc.scalar.activation(trash[:], x2_all[:, w, :], AF.Identity,
                                         accum_out=sx_all[:, w:w + 1])

                prev = None       # (w, psV, psA) of the previous window
                prev_attin = None  # (w, attin) pending tail_b
                for w in range(NW):
                    NT = nts[w]
                    TW = NT * P
                    eo = sum(n * P // 16 for n in nts[:w])
                    to = int(toff[w])
                    if w in edge_tiles:
                        s12_t, rb_t = edge_tiles.pop(w)
                    else:
                        s12_t = big.tile([P, NTmax, 2, P], BF16, tag="s12", bufs=3)
                        nc.sync.dma_start(s12_t[:, 0:2, :, :],
                                          s12.ap()[:, to:to + 2, :, :])
                        nc.sync.dma_start(s12_t[:, 2:NT, :, :],
                                          s12.ap()[:, to + 2:to + NT, :, :])
                        rb_t = big.tile([P, NTmax, 12], BF16, tag="rb", bufs=3)
                        nc.sync.dma_start(rb_t[:, 0:NT, :],
                                          relbias.ap()[:, to:to + NT, :])
                    idx_t = big.tile([P, NTmax * P // 16], I16, tag="idx", bufs=3)
                    nc.sync.dma_start(idx_t[:, 0:TW // 16],
                                      eidx.ap()[:, eo:eo + TW // 16])
                    kv_g = big.tile([P, NTmax, KVB], U8, tag="kv", bufs=3)
                    # split each gather into quarter-gathers alternating the
                    # two SWDGE queues: the first chunk lands earlier (qe for
                    # the first tiles starts sooner) and rings stay pipelined.
                    QC = 1  # tiles per gather chunk
                    for ci, c0 in enumerate(range(0, NT, QC)):
                        cb = min(QC, NT - c0)
                        nc.gpsimd.dma_gather(
                            out_ap=kv_g[:, c0:c0 + cb, :], in_ap=kvt[:],
                            idxs_ap=idx_t[:, c0 * P // 16:(c0 + cb) * P // 16],
                            num_idxs=cb * P, num_idxs_reg=cb * P, elem_size=KVB,
                            single_packet=False, queue_num=ci % 2)

                    psV = ppsV.tile([P, 512], F32, tag="psV")
                    psA = ppsA.tile([P, 48], F32, tag="psA")
                    ngrp = (NT + GS - 1) // GS
                    for gi, t0 in enumerate(range(0, NT, GS)):
                        tb = min(GS, NT - t0)
                        qe = pqe.tile([P, GS, C], F32, tag="qe")
                        for d_ in range(tb):
                            nc.tensor.matmul(qe[:, d_, :],
                                             lhsT=s12_t[:, t0 + d_, 1, :],
                                             rhs=q_sbuf[:, w, :], start=True, stop=False)
                            nc.tensor.matmul(qe[:, d_, :], lhsT=ident_f8[:],
                                             rhs=kv_g[:, t0 + d_, 0:C].bitcast(FP8),
                                             start=False, stop=True)
                        if gi == 1 and prev is not None:
                            if prev_attin is not None:
                                tail_b(*prev_attin)
                            prev_attin = (prev[0], tail_a(*prev))
                            prev = None
                        dsq = wk.tile([P, GS, C], BF16, tag="dsq")
                        nc.scalar.activation(dsq[:, 0:tb, :], qe[:, 0:tb, :], AF.Square)
                        # staged-halving reduce: two bf16 2x-mode adds, then a
                        # short TensorReduce (TensorReduce has no fast mode).
                        d4 = dsq[:, 0:tb, :].rearrange("p t (h j d) -> p t h j d", h=H, j=2)
                        h1 = wk.tile([P, GS, H, DH // 2], BF16, tag="h1")
                        nc.vector.tensor_add(h1[:, 0:tb, :, :], d4[:, :, :, 0, :], d4[:, :, :, 1, :])
                        h14 = h1[:, 0:tb, :, :].rearrange("p t h (j d) -> p t h j d", j=2)
                        h2 = wk.tile([P, GS, H, DH // 4], BF16, tag="h2")
                        nc.vector.tensor_add(h2[:, 0:tb, :, :], h14[:, :, :, 0, :], h14[:, :, :, 1, :])
                        s8 = wk.tile([P, GS, H], F32, tag="s8")
                        nc.vector.reduce_sum(
                            s8[:, 0:tb, :], h2[:, 0:tb, :, :], axis=AX.X)
                        sc = wk.tile([P, GS, H], F32, tag="sc")
                        nc.vector.scalar_tensor_tensor(
                            out=sc[:, 0:tb, :], in0=s8[:, 0:tb, :], scalar=-inv_s,
                            in1=rb_t[:, t0:t0 + tb, 4:12], op0=ALU.mult, op1=ALU.add)
                        eaux = wk.tile([P, GS, 48], BF16, tag="eaux")
                        nc.scalar.activation(
                            eaux[:, 0:tb, 0:16].rearrange("p t (h j) -> p t h j", h=H),
                            sc[:, 0:tb, :].unsqueeze(3).broadcast_to([P, tb, H, 2]),
                            AF.Exp)
                        pev = wk.tile([P, GS, C], BF16, tag="pev")
                        for d_ in range(tb):
                            nc.vector.tensor_mul(
                                pev[:, d_, :].rearrange("p (h a j) -> p h a j", h=H, j=2),
                                eaux[:, d_, 0:16].rearrange("p (h j) -> p h j", h=H)
                                    .unsqueeze(2).broadcast_to([P, H, DH // 2, 2]),
                                kv_g[:, t0 + d_, C:KVB].bitcast(BF16)
                                    .rearrange("p (h a j) -> p h a j", h=H, j=2))
                        nc.vector.tensor_mul(
                            eaux[:, 0:tb, 16:48].rearrange("p t (h a) -> p t h a", h=H),
                            eaux[:, 0:tb, 0:16].rearrange("p t (h j) -> p t h j", h=H)[:, :, :, 0:1]
                                .broadcast_to([P, tb, H, 4]),
                            rb_t[:, t0:t0 + tb, 0:4].unsqueeze(2)
                                .broadcast_to([P, tb, H, 4]))
                        for d_ in range(tb):
                            t = t0 + d_
                            nc.tensor.matmul(psV[:], lhsT=s12_t[:, t, 0, :],
                                             rhs=pev[:, d_, :],
                                             start=(t == 0), stop=False)
                            nc.tensor.matmul(psA[:], lhsT=s12_t[:, t, 0, :],
                                             rhs=eaux[:, d_, :],
                                             start=(t == 0), stop=(t == NT - 1))

                    prev = (w, psV, psA)
                if prev is not None:
                    if prev_attin is not None:
                        tail_b(*prev_attin)
                    prev_attin = (prev[0], tail_a(*prev))
                if prev_attin is not None:
                    tail_b(*prev_attin)

            # ============ P4: LN2 + FFN ============
            if "p4" not in _SKIP:
             with tc.tile_pool(name="p4", bufs=4) as wk, \
                 tc.tile_pool(name="p4tp", bufs=4, space="PSUM") as ptp, \
                 tc.tile_pool(name="f1ps", bufs=2, space="PSUM") as pps1, \
                 tc.tile_pool(name="f2ps", bufs=2, space="PSUM") as pps2:
                # mean/var from the Act-accumulated sums: mean = sx/C,
                # var = sx2/C - mean^2; one batched Sqrt (one table switch).
                mean_all = wk.tile([P, NW], F32, tag="mean_all")
                nc.vector.tensor_scalar_mul(mean_all[:], sx_all[:], 1.0 / C)
                msq = wk.tile([P, NW], F32, tag="msq")
                nc.vector.tensor_mul(msq[:], mean_all[:], mean_all[:])
                var_all = wk.tile([P, NW], F32, tag="var_all")
                nc.vector.scalar_tensor_tensor(
                    out=var_all[:], in0=sx2_all[:], scalar=1.0 / C,
                    in1=msq[:], op0=ALU.mult, op1=ALU.subtract)
                sd_all = wk.tile([P, NW], F32, tag="sd_all")
                nc.scalar.activation(sd_all[:], var_all[:], AF.Sqrt,
                                     bias=eps_t[:], scale=1.0)
                rs_all = wk.tile([P, NW], F32, tag="rs_all")
                nc.vector.reciprocal(rs_all[:], sd_all[:])
                nmr_all = wk.tile([P, NW], F32, tag="nmr_all")
                nc.vector.scalar_tensor_tensor(
                    out=nmr_all[:], in0=mean_all[:], scalar=-1.0,
                    in1=rs_all[:], op0=ALU.mult, op1=ALU.mult)
                def ffn_chunk(rc):
                    for ht in range(HID // P):
                        ps = pps1.tile([P, 512], F32, tag="ps1")
                        for ko in range(0, C // P, 2):
                            nc.tensor.matmul(
                                ps[:], lhsT=w1_s[:, ko:ko + 2, ht * P:(ht + 1) * P],
                                rhs=zt_sbuf[:, ko:ko + 2, rc * 512:(rc + 1) * 512],
                                start=(ko == 0), stop=(ko == C // P - 2),
                                perf_mode=mybir.MatmulPerfMode.DoubleRow)
                        nc.scalar.activation(
                            hT_sbuf[:, ht, rc * 512:(rc + 1) * 512], ps[:],
                            AF.Gelu_apprx_tanh, bias=b1c_s[:, ht:ht + 1], scale=1.0 / WS)
                    for m in range(rc * 4, rc * 4 + 4):
                        ps = pps2.tile([P, C], F32, tag="ps2")
                        for ht in range(0, HID // P, 2):
                            nc.tensor.matmul(ps[:], lhsT=hT_sbuf[:, ht:ht + 2, m * P:(m + 1) * P],
                                             rhs=w2_s[:, ht:ht + 2, :],
                                             start=(ht == 0), stop=False,
                                             perf_mode=mybir.MatmulPerfMode.DoubleRow)
                        nc.tensor.matmul(ps[:], lhsT=ones_s[:], rhs=brow_s[0:1, 2 * C:3 * C],
                                         start=False, stop=True)
                        if m % 2 == 0:
                            yt = wk.tile([P, 2, C], F32, tag="y", bufs=2)
                        nc.vector.scalar_tensor_tensor(
                            out=yt[:, m % 2, :], in0=ps[:], scalar=1.0 / WS,
                            in1=x2_all[:, m, :], op0=ALU.mult, op1=ALU.add)
                        if m % 2 == 1:
                            nc.sync.dma_start(y_t[:, m - 1:m + 1, :], yt[:])

                # LN2 windows with FFN chunks interleaved after every 4th.
                for m in range(NW):
                    zf = wk.tile([P, C], BF16, tag="ln_zf")
                    nc.vector.tensor_scalar(
                        out=zf[:], in0=x2_all[:, m, :],
                        scalar1=rs_all[:, m:m + 1], scalar2=nmr_all[:, m:m + 1],
                        op0=ALU.mult, op1=ALU.add)
                    tp = ptp.tile([P, C], BF16, tag="tp")
                    for c4 in range(C // P):
                        nc.tensor.transpose(tp[:, c4 * P:(c4 + 1) * P],
                                            zf[:, c4 * P:(c4 + 1) * P], ident[:])
                    if m % 2 == 0:
                        nc.scalar.activation(
                            zt_sbuf[:, :, m * P:(m + 1) * P],
                            tp[:].rearrange("p (c4 q) -> p c4 q", q=P), AF.Identity)
                    else:
                        nc.vector.tensor_copy(
                            zt_sbuf[:, :, m * P:(m + 1) * P],
                            tp[:].rearrange("p (c4 q) -> p c4 q", q=P))
                    if m % 4 == 3:
                        ffn_chunk(m // 4)

    nc.compile()
    return nc


def _prep(inputs):
    row = np.asarray(inputs["row_index"]).astype(np.int64).ravel()
    col = np.asarray(inputs["col_index"]).astype(np.int64).ravel()
    tcol = np.asarray(inputs["to_col_index"]).astype(np.int64).ravel()
    bias = np.asarray(inputs["pos_att_bias"], dtype=np.float32)
    dist = np.asarray(inputs["dist"], dtype=np.float32).ravel()
    pos = np.asarray(inputs["pos"], dtype=np.float32)
    cpos = np.asarray(inputs["col_pos"], dtype=np.float32)

    # ---- balance rows into 128 bins (8 cores x 16 windows, 128 rows each) ---
    import heapq
    NB = NCORES * NW
    cnt = np.bincount(row, minlength=L)
    order_r = np.argsort(-cnt, kind="stable")
    heap = [(0, 0, b) for b in range(NB)]
    heapq.heapify(heap)
    bin_rows = [[] for _ in range(NB)]
    bin_sum = np.zeros(NB, np.int64)
    for r in order_r:
        popped = []
        while True:
            s, n, b = heapq.heappop(heap)
            if n < P:
                break
            popped.append((s, n, b))
        for x_ in popped:
            heapq.heappush(heap, x_)
        bin_rows[b].append(int(r))
        bin_sum[b] = s + cnt[r]
        heapq.heappush(heap, (int(bin_sum[b]), n + 1, b))

    # snake-assign bins to cores by edge-count rank; windows sorted descending
    # within each core so window j's count is similar across cores.
    rk = np.argsort(-bin_sum)
    core_bins = [[] for _ in range(NCORES)]
    for i, b in enumerate(rk):
        core_bins[i % NCORES].append(int(b))
    for c in range(NCORES):
        core_bins[c].sort(key=lambda b: -int(bin_sum[b]))
    counts = np.array([[bin_sum[b] for b in core_bins[c]] for c in range(NCORES)])
    nts = tuple(int(x) for x in np.ceil(counts.max(axis=0) / P).astype(int))

    # global row permutation: new row (c*RS + w*P + slot) = old row
    perm = np.empty(L, np.int64)
    for c in range(NCORES):
        for w in range(NW):
            b = core_bins[c][w]
            perm[c * RS + w * P:c * RS + (w + 1) * P] = bin_rows[b]
    inv_perm = np.empty(L, np.int64)
    inv_perm[perm] = np.arange(L)

    new_row = inv_perm[row]   # position of each edge's target row
    new_col = inv_perm[col]   # position of each edge's source col in permuted kvt

    TOT = sum(nts)
    E16 = sum(n * P // 16 for n in nts)
    toff = np.concatenate([[0], np.cumsum(nts)]).astype(int)
    eoff = np.concatenate([[0], np.cumsum([n * P // 16 for n in nts])]).astype(int)

    eidx_h = np.zeros((NCORES, P, E16), np.int16)
    rb_h = np.zeros((NCORES, P, TOT, 12), np.float32)
    rb_h[:, :, :, 4:12] = -1e4
    s12_h = np.zeros((NCORES, P, TOT, 2, P), np.float32)

    gw_all = new_row // P  # global window id (0..127) per edge
    order_e = np.argsort(gw_all, kind="stable")
    gw_s = gw_all[order_e]
    starts = np.searchsorted(gw_s, np.arange(NB + 1))
    for gw in range(NB):
        c, w = divmod(gw, NW)
        sl = order_e[starts[gw]:starts[gw + 1]]
        n = len(sl)
        if n == 0:
            continue
        TWw = nts[w] * P
        assert n <= TWw, (n, TWw)
        erows = (new_row[sl] - gw * P).astype(np.int64)
        ecols = new_col[sl]
        j = np.arange(n)
        wrap = np.zeros((16, TWw // 16), np.int16)
        wrap[j % 16, j // 16] = ecols.astype(np.int16)
        eidx_h[c, :, eoff[w]:eoff[w + 1]] = np.tile(wrap, (8, 1))
        t_of = toff[w] + j // P
        e_of = j % P
        rb_h[c, e_of, t_of, 0:3] = (cpos[tcol[sl]] - pos[row[sl]]) / dist[sl][:, None]
        rb_h[c, e_of, t_of, 3] = 1.0
        rb_h[c, e_of, t_of, 4:12] = bias[sl]
        s12_h[c, e_of, t_of, 0, erows] = 1.0   # s1: edge -> row scatter
        s12_h[c, erows, t_of, 1, e_of] = 1.0   # s2: row -> edge expand
    import ml_dtypes
    bf = ml_dtypes.bfloat16
    return nts, perm, eidx_h, rb_h.astype(bf), s12_h.astype(bf)


def kernel(**inputs):
    import ml_dtypes
    bf = ml_dtypes.bfloat16
    x = np.asarray(inputs["x"], dtype=np.float32)
    nts, perm, eidx_h, rb_h, s12_h = _prep(inputs)
    if nts not in _cache:
        _cache[nts] = _build(nts)
    nc = _cache[nts]

    f32 = lambda k: np.asarray(inputs[k], np.float32)
    g1, b1l = f32("ln1_g"), f32("ln1_b")
    g2, b2l = f32("ln2_g"), f32("ln2_b")
    Wq, Wk, Wv, Wo = f32("Wq"), f32("Wk"), f32("Wv"), f32("Wo")
    # Fold LN affine into the following matmuls; fold bk into bq (only the
    # difference q-k matters) and bv into bo (sum_e alpha = 1 per head).
    Wq_, Wk_, Wv_ = g1[:, None] * Wq, g1[:, None] * Wk, g1[:, None] * Wv
    bq_ = (b1l @ Wq + f32("bq")) - (b1l @ Wk + f32("bk"))
    bo_ = (b1l @ Wv + f32("bv")) @ Wo + f32("bo")
    W1_ = g2[:, None] * f32("W1")
    b1_ = b2l @ f32("W1") + f32("b1")
    import ml_dtypes as _md
    f8 = _md.float8_e4m3
    WS = 64.0
    # Wk negated: the kernel stores k pre-negated for the PE qe-k accumulate.
    w_qkv = (np.concatenate([Wq_, -Wk_, Wv_], axis=1) * WS).astype(f8)

    wv4 = np.concatenate([f32("Wvec"), f32("bvec")[None, :]], axis=0)
    w_vec4 = np.zeros((32, C), np.float32)
    for h in range(H):
        w_vec4[4 * h:4 * h + 4, h * DH:(h + 1) * DH] = wv4[:, h * DH:(h + 1) * DH]

    brows = np.zeros((1, 4 * C), np.float32)
    brows[0, 0:C] = bq_
    brows[0, C:2 * C] = bo_
    brows[0, 2 * C:3 * C] = f32("b2")
    b1_col = np.ascontiguousarray(b1_.reshape(HID // P, P).T)

    xp = x[perm]
    in_maps = []
    for c in range(NCORES):
        in_maps.append(dict(
            x_in=np.ascontiguousarray(xp[c * RS:(c + 1) * RS]),
            w_qkv=w_qkv, w_o=(Wo * WS).astype(f8),
            w_1=(W1_ * WS).astype(f8), w_2=(f32("W2") * WS).astype(f8),
            w_vec4=w_vec4.astype(bf), b1_col=b1_col,
            brows=(brows * WS).astype(f8),
            eidx=eidx_h[c], relbias=rb_h[c], s12=s12_h[c],
        ))
    _last["nc"] = nc
    _last["in_maps"] = in_maps
    res = run_bass_kernel_spmd(nc, in_maps, list(range(NCORES)))
    global _last_res
    _last_res = res
    yp = np.concatenate([res.results[c]["y_out"] for c in range(NCORES)], axis=0)
    y = np.empty_like(yp)
    y[perm] = yp
    return np.asarray(y, np.float32)


_last = {}
_last_res = None


# revision 85
# speedup vs baseline: 1.2925x; 1.0037x over previous
"""Trainium2 Bass kernel for nn_EncoderLayer_88476326298146 (sparse graph attention).

Row-sharded across 8 NeuronCores with host-side load balancing: all L rows are
LPT-packed into 128 bins (8 cores x 16 windows, exactly 128 rows each) so edge
counts per window are near-uniform; per-window tile counts (nts) are baked into
the build. k/v (k fp8 negated via host-negated Wk, v bf16; 1.5KB/row) are replicated via
AllGather; per-edge col features come from per-tile dma_gather chunks
alternating the two SWDGE queues (early chunks land sooner, and one full-window
gather would fill a whole 1024-descriptor ring).

- LN affine folded into following weights host-side; biases via rank-1 ones-row
  matmuls on the PE.
- diff = q_row - k_col on the PE (s2^T@q then accumulate ident@(-k)); square on
  Act from PSUM; per-head reduce = two bf16 2x-mode halving adds + short reduce.
- exp emitted as bf16 PAIRS (eaux[...,0:16]) and shared by the alpha*v multiply
  (DVE 2x) and the aux (den/rel) matmul.
- segment softmax with m=0; segment sums via host-built one-hot matmuls.
- FFN1 produced transposed with gelu+bias fused on Act; FFN2 consumes h^T as
  lhsT. LN2 uses one batched Sqrt so the act table switches only once.
- DMA batching: s1+s2 in one tensor, rel+bias in one bf16 tensor, k+v in one
  store per window; ident/ones/eps generated on-chip.
"""
import os
import numpy as np

import concourse.bass as bass
import concourse.bacc as bacc
import concourse.mybir as mybir
import concourse.tile as tile
from concourse.bass_utils import run_bass_kernel_spmd
from concourse.library_config import mlp as mlp_lib

L, E, SP, C, H, DH, HID = 16384, 131072, 20000, 512, 8, 64, 1024
NCORES = 8
RS = L // NCORES
NW = RS // 128
P = 128
F32 = mybir.dt.float32
BF16 = mybir.dt.bfloat16
I16 = mybir.dt.int16
FP8 = mybir.dt.float8e4
U8 = mybir.dt.uint8
KVB = 3 * C  # kv row bytes: k fp8 (C) + v bf16 (2C)
WS = 64.0  # weight pre-scale (fp8 subnormal avoidance); descaled in Act casts
AF = mybir.ActivationFunctionType
ALU = mybir.AluOpType
AX = mybir.AxisListType

_cache = {}
_SKIP = set(os.environ.get("KSKIP", "").split(","))


def _build(nts):
    if isinstance(nts, int):
        nts = (nts // P,) * NW
    nts = tuple(int(n) for n in nts)
    assert len(nts) == NW
    NTmax = max(nts)
    TOT = sum(nts)             # total tiles across windows
    E16 = sum(n * P // 16 for n in nts)  # eidx columns
    toff = np.concatenate([[0], np.cumsum(nts)]).astype(int)
    GS = 2  # tiles per score group (PSUM: GS banks for qe)
    inv_s = 1.0 / float(np.sqrt(DH))
    nc = bacc.Bacc("TRN2", target_bir_lowering=False, debug=False, num_devices=NCORES,
                   num_swdge_queues=2)

    x_in = nc.dram_tensor("x_in", [RS, C], F32, kind="ExternalInput")
    w_qkv = nc.dram_tensor("w_qkv", [C, 3 * C], FP8, kind="ExternalInput")
    w_o = nc.dram_tensor("w_o", [C, C], FP8, kind="ExternalInput")
    w_1 = nc.dram_tensor("w_1", [C, HID], FP8, kind="ExternalInput")
    w_2 = nc.dram_tensor("w_2", [HID, C], FP8, kind="ExternalInput")
    w_vec4 = nc.dram_tensor("w_vec4", [32, C], BF16, kind="ExternalInput")
    b1_col = nc.dram_tensor("b1_col", [P, HID // P], F32, kind="ExternalInput")
    brows = nc.dram_tensor("brows", [1, 4 * C], FP8, kind="ExternalInput")
    eidx = nc.dram_tensor("eidx", [P, E16], I16, kind="ExternalInput")
    relbias = nc.dram_tensor("relbias", [P, TOT, 12], BF16, kind="ExternalInput")
    s12 = nc.dram_tensor("s12", [P, TOT, 2, P], BF16, kind="ExternalInput")
    y_out = nc.dram_tensor("y_out", [RS, C], F32, kind="ExternalOutput")

    x_t = x_in.ap().rearrange("(m p) n -> p m n", p=P)
    y_t = y_out.ap().rearrange("(m p) n -> p m n", p=P)

    with tile.TileContext(nc) as tc:
        with tc.tile_pool(name="dram", bufs=1, space="DRAM") as dram, \
             tc.tile_pool(name="const", bufs=1) as const, \
             tc.tile_pool(name="big", bufs=2) as big:
            nc.gpsimd.load_library(mlp_lib)

            # x loads first: LN(0) is the startup critical path.
            x_all = const.tile([P, NW, C], F32)
            nc.sync.dma_start(x_all[:, 0, :], x_t[:, 0, :])
            nc.sync.dma_start(x_all[:, 1, :], x_t[:, 1, :])
            nc.sync.dma_start(x_all[:, 2:4, :], x_t[:, 2:4, :])

            # on-chip constants: ident[p,j] = (j - p == 0), ones, eps (no DMAs
            # -> less HWDGE descriptor serialization at startup).
            iota_d = const.tile([P, P], I16)
            nc.gpsimd.iota(iota_d[:], pattern=[[1, P]], base=0, channel_multiplier=-1)
            ident = const.tile([P, P], BF16)
            nc.vector.tensor_scalar(out=ident[:], in0=iota_d[:], scalar1=0,
                                    scalar2=None, op0=ALU.is_equal)
            ident_f8 = const.tile([P, P], FP8)
            nc.vector.tensor_scalar(out=ident_f8[:], in0=iota_d[:], scalar1=0,
                                    scalar2=None, op0=ALU.is_equal)
            ones_s = const.tile([1, P], FP8)
            nc.vector.memset(ones_s[:], 1.0)
            eps_t = const.tile([P, 1], F32)
            nc.vector.memset(eps_t[:], 1e-5)


            brow_s = const.tile([1, 4 * C], FP8)
            nc.sync.dma_start(brow_s[:], brows.ap())
            wvec_s = const.tile([32, C], BF16)
            nc.sync.dma_start(wvec_s[:], w_vec4.ap())
            b1c_s = const.tile([P, HID // P], F32)
            nc.sync.dma_start(b1c_s[:], b1_col.ap())

            # weight prefetch (Pool queue; overlaps P1)
            wqkv_s = const.tile([P, C // P, 3 * C], FP8, name="wqkv")
            nc.gpsimd.dma_start(wqkv_s[:], w_qkv.ap().rearrange("(ko p) n -> p ko n", p=P))
            wo_s = const.tile([P, C // P, C], FP8, name="wo")
            nc.gpsimd.dma_start(wo_s[:], w_o.ap().rearrange("(ko p) n -> p ko n", p=P))
            for xm in range(4, 16, 2):
                nc.sync.dma_start(x_all[:, xm:xm + 2, :], x_t[:, xm:xm + 2, :])
            # FFN weights are needed only in the tail: load them after x.
            w1_s = const.tile([P, C // P, HID], FP8, name="w1")
            nc.gpsimd.dma_start(w1_s[:], w_1.ap().rearrange("(ko p) n -> p ko n", p=P))
            w2_s = const.tile([P, HID // P, C], FP8, name="w2")
            nc.gpsimd.dma_start(w2_s[:], w_2.ap().rearrange("(ko p) n -> p ko n", p=P))

            x2_all = const.tile([P, NW, C], BF16)
            sx_all = const.tile([P, NW], F32)    # per-window sum(x2) (LN2)
            sx2_all = const.tile([P, NW], F32)   # per-window sum(x2^2)
            zt_sbuf = const.tile([P, C // P, RS], FP8)
            q_sbuf = const.tile([P, NW, C], BF16)
            hT_sbuf = const.tile([P, HID // P, RS], FP8)

            kv_shard = dram.tile([RS, KVB], U8)
            if "ag" not in _SKIP:
                kvt = dram.tile([L, KVB], U8, addr_space="Shared")
            else:
                kvt = dram.tile([L, KVB], U8)

            # Edge-phase loads for the first windows issued BEFORE P1 so they
            # prefetch during P1 (the SP ring is in-order; emitting them after
            # P1's kv stores would delay them to the end of P1).
            edge_tiles = {}
            for w in range(3):
                NT = nts[w]
                to = int(toff[w])
                s12_t = big.tile([P, NTmax, 2, P], BF16, tag="s12", bufs=3)
                nc.sync.dma_start(s12_t[:, 0:NT, :, :],
                                  s12.ap()[:, to:to + NT, :, :])
                rb_t = big.tile([P, NTmax, 12], BF16, tag="rb", bufs=3)
                nc.sync.dma_start(rb_t[:, 0:NT, :],
                                  relbias.ap()[:, to:to + NT, :])
                edge_tiles[w] = (s12_t, rb_t)

            # ---------- LN helper: stats + normalized bf16 z (no affine) ----
            def ln_win(wk, ptp, src, m, copy_eng="v"):
                stats = wk.tile([P, 6], F32, tag="ln_st")
                nc.vector.bn_stats(stats[:], src)
                mv = wk.tile([P, 2], F32, tag="ln_mv")
                nc.vector.bn_aggr(mv[:], stats[:])
                sd = wk.tile([P, 1], F32, tag="ln_sd")
                nc.scalar.activation(sd[:], mv[:, 1:2], AF.Sqrt, bias=eps_t[:], scale=1.0)
                rs_ = wk.tile([P, 1], F32, tag="ln_rs")
                nc.vector.reciprocal(rs_[:], sd[:])
                nmr = wk.tile([P, 1], F32, tag="ln_nmr")
                nc.vector.scalar_tensor_tensor(
                    out=nmr[:], in0=mv[:, 0:1], scalar=-1.0, in1=rs_[:],
                    op0=ALU.mult, op1=ALU.mult)
                zf = wk.tile([P, C], BF16, tag="ln_zf")
                nc.scalar.activation(zf[:], src, AF.Identity, bias=nmr[:], scale=rs_[:])
                tp = ptp.tile([P, C], BF16, tag="tp")
                for c4 in range(C // P):
                    nc.tensor.transpose(tp[:, c4 * P:(c4 + 1) * P],
                                        zf[:, c4 * P:(c4 + 1) * P], ident[:])
                if copy_eng == "a":
                    nc.scalar.activation(
                        zt_sbuf[:, :, m * P:(m + 1) * P],
                        tp[:].rearrange("p (c4 q) -> p c4 q", q=P), AF.Identity)
                elif copy_eng == "g":
                    nc.gpsimd.tensor_copy(
                        zt_sbuf[:, :, m * P:(m + 1) * P],
                        tp[:].rearrange("p (c4 q) -> p c4 q", q=P))
                else:
                    nc.vector.tensor_copy(
                        zt_sbuf[:, :, m * P:(m + 1) * P],
                        tp[:].rearrange("p (c4 q) -> p c4 q", q=P))

            # ============ P1: LN1 + QKV ============
            if "p1" not in _SKIP:
             with tc.tile_pool(name="p1", bufs=4) as wk, \
                 tc.tile_pool(name="p1tp", bufs=4, space="PSUM") as ptp, \
                 tc.tile_pool(name="p1ps", bufs=4, space="PSUM") as pps:
                kv_sh_t = kv_shard[:].rearrange("(m p) n -> p m n", p=P)
                def qkv_part(m):
                    kvb = wk.tile([P, KVB], U8, tag="kvb")
                    for nb in range(3):
                        ps = pps.tile([P, C], F32, tag="ps")
                        for ko in range(0, C // P, 2):
                            nc.tensor.matmul(
                                ps[:],
                                lhsT=zt_sbuf[:, ko:ko + 2, m * P:(m + 1) * P],
                                rhs=wqkv_s[:, ko:ko + 2, nb * C:(nb + 1) * C],
                                start=(ko == 0), stop=(ko == C // P - 2 and nb != 0),
                                perf_mode=mybir.MatmulPerfMode.DoubleRow)
                        if nb == 0:
                            nc.tensor.matmul(ps[:], lhsT=ones_s[:], rhs=brow_s[0:1, 0:C],
                                             start=False, stop=True)
                            nc.scalar.activation(q_sbuf[:, m, :], ps[:], AF.Identity,
                                                 scale=1.0 / WS)
                        else:
                            # k stored negated (Wk negated host-side) so the edge
                            # phase accumulates qe + (-k) on the PE via ident.
                            # k cast on Act, v on DVE; one combined DMA.
                            if nb == 1:
                                nc.scalar.mul(kvb[:, 0:C].bitcast(FP8), ps[:], 1.0 / WS)
                            else:
                                nc.vector.tensor_scalar_mul(
                                    kvb[:, C:KVB].bitcast(BF16), ps[:], 1.0 / WS)
                    nc.sync.dma_start(kv_sh_t[:, m, :], kvb[:])

                # software-pipelined: window m's QKV emitted after window
                # m+1's LN so the LN chain overlaps the previous QKV.
                for m in range(NW):
                    # zt copies alternate Act/DVE; nothing from P1 runs on the
                    # Pool queue, so gather(0)'s descriptor-gen is not blocked
                    # behind P1 (Pool is in-order).
                    ln_win(wk, ptp, x_all[:, m, :], m,
                           copy_eng=("a" if m % 2 == 0 else "v"))
                    if m > 0:
                        qkv_part(m - 1)
                qkv_part(NW - 1)

            # ============ P2: AllGather ============
            if "ag" not in _SKIP:
                nc.gpsimd.collective_compute(
                    "AllGather", ALU.bypass, replica_groups=[list(range(NCORES))],
                    ins=[kv_shard[:].opt()], outs=[kvt[:].opt()])

            # ============ P3: edge windows + Wo + residual ============
            # `big` lives at top level so s12/idx DMAs and gathers are not
            # WAR-serialized behind P1's SBUF.
            if "edge" not in _SKIP:
             with tc.tile_pool(name="ew", bufs=5) as wk, \
                 tc.tile_pool(name="pqe", bufs=1, space="PSUM") as pqe, \
                 tc.tile_pool(name="ppsV", bufs=2, space="PSUM") as ppsV, \
                 tc.tile_pool(name="ppsA", bufs=2, space="PSUM") as ppsA, \
                 tc.tile_pool(name="ptpc", bufs=1, space="PSUM") as ptpc, \
                 tc.tile_pool(name="p5ps", bufs=1, space="PSUM") as p5ps:
                # tail(w): den -> rden -> anr -> an_ts -> wvec-mm -> attin ->
                # transpose -> Wo-mm -> x2.  Emitted one window late (split in
                # two parts interleaved with window w+1's groups) so its long
                # cross-engine latency chain overlaps the next window's bulk
                # work instead of stalling the in-order engine queues.
                def tail_a(w, psV, psA):
                    den = wk.tile([P, 16], F32, tag="den")
                    nc.vector.tensor_scalar_max(den[:], psA[:, 0:16], 1e-30)
                    rden = wk.tile([P, 16], F32, tag="rden")
                    nc.vector.reciprocal(rden[:], den[:])
                    # fold the (unnormalized) Wvec term into psV: w_vec4 is
                    # head-block-diagonal, so per-(row,head) rden factors
                    # through the sum.
                    anr = wk.tile([P, 32], BF16, tag="anr")
                    nc.scalar.activation(anr[:], psA[:, 16:48], AF.Identity)
                    tpc = ptpc.tile([P, C], BF16, tag="tpc")
                    nc.tensor.transpose(tpc[0:32, 0:P], anr[:], ident[:])
                    an_ts = wk.tile([32, P], BF16, tag="an_ts")
                    nc.scalar.activation(an_ts[:], tpc[0:32, 0:P], AF.Identity)
                    nc.tensor.matmul(psV[:], lhsT=an_ts[:], rhs=wvec_s[:],
                                     start=False, stop=True)
                    attin = wk.tile([P, C], BF16, tag="attin")
                    nc.vector.tensor_mul(
                        attin[:].rearrange("p (h d) -> p h d", h=H),
                        psV[:].rearrange("p (h d) -> p h d", h=H),
                        rden[:].rearrange("p (h j) -> p h j", h=H)[:, :, 0:1]
                            .broadcast_to([P, H, DH]))
                    return attin

                def tail_b(w, attin):
                    tpa = ptpc.tile([P, C], BF16, tag="tpc")
                    for c4 in range(C // P):
                        nc.tensor.transpose(tpa[:, c4 * P:(c4 + 1) * P],
                                            attin[:, c4 * P:(c4 + 1) * P], ident[:])
                    at_sb = wk.tile([P, C // P, P], FP8, tag="at_sb")
                    nc.scalar.activation(
                        at_sb[:], tpa[:].rearrange("p (c4 q) -> p c4 q", q=P),
                        AF.Identity)
                    x2ps = p5ps.tile([P, C], F32, tag="p5")
                    for ko in range(0, C // P, 2):
                        nc.tensor.matmul(x2ps[:], lhsT=at_sb[:, ko:ko + 2, :],
                                         rhs=wo_s[:, ko:ko + 2, :],
                                         start=(ko == 0), stop=False,
                                         perf_mode=mybir.MatmulPerfMode.DoubleRow)
                    nc.tensor.matmul(x2ps[:], lhsT=ones_s[:], rhs=brow_s[0:1, C:2 * C],
                                     start=False, stop=True)
                    nc.vector.scalar_tensor_tensor(
                        out=x2_all[:, w, :], in0=x2ps[:], scalar=1.0 / WS,
                        in1=x_all[:, w, :], op0=ALU.mult, op1=ALU.add)
                    # LN2 stats via the Act accumulator (Act has slack in the
                    # edge phase; keeps the bn_stats chain out of the tail).
                    trash = wk.tile([P, C], BF16, tag="trash")
                    nc.scalar.activation(trash[:], x2_all[:, w, :], AF.Square,
                                         accum_out=sx2_all[:, w:w + 1])
                    nc.scalar.activation(trash[:], x2_all[:, w, :], AF.Identity,
                                         accum_out=sx_all[:, w:w + 1])

                prev = None       # (w, psV, psA) of the previous window
                prev_attin = None  # (w, attin) pending tail_b
                for w in range(NW):
                    NT = nts[w]
                    TW = NT * P
                    eo = sum(n * P // 16 for n in nts[:w])
                    to = int(toff[w])
                    if w in edge_tiles:
                        s12_t, rb_t = edge_tiles.pop(w)
                    else:
                        s12_t = big.tile([P, NTmax, 2, P], BF16, tag="s12", bufs=3)
                        nc.sync.dma_start(s12_t[:, 0:2, :, :],
                                          s12.ap()[:, to:to + 2, :, :])
                        nc.sync.dma_start(s12_t[:, 2:NT, :, :],
                                          s12.ap()[:, to + 2:to + NT, :, :])
                        rb_t = big.tile([P, NTmax, 12], BF16, tag="rb", bufs=3)
                        nc.sync.dma_start(rb_t[:, 0:NT, :],
                                          relbias.ap()[:, to:to + NT, :])
                    idx_t = big.tile([P, NTmax * P // 16], I16, tag="idx", bufs=3)
                    nc.sync.dma_start(idx_t[:, 0:TW // 16],
                                      eidx.ap()[:, eo:eo + TW // 16])
                    kv_g = big.tile([P, NTmax, KVB], U8, tag="kv", bufs=3)
                    # split each gather into quarter-gathers alternating the
                    # two SWDGE queues: the first chunk lands earlier (qe for
                    # the first tiles starts sooner) and rings stay pipelined.
                    QC = 1  # tiles per gather chunk
                    for ci, c0 in enumerate(range(0, NT, QC)):
                        cb = min(QC, NT - c0)
                        nc.gpsimd.dma_gather(
                            out_ap=kv_g[:, c0:c0 + cb, :], in_ap=kvt[:],
                            idxs_ap=idx_t[:, c0 * P // 16:(c0 + cb) * P // 16],
                            num_idxs=cb * P, num_idxs_reg=cb * P, elem_size=KVB,
                            single_packet=False, queue_num=ci % 2)

                    psV = ppsV.tile([P, 512], F32, tag="psV")
                    psA = ppsA.tile([P, 48], F32, tag="psA")
                    ngrp = (NT + GS - 1) // GS
                    for gi, t0 in enumerate(range(0, NT, GS)):
                        tb = min(GS, NT - t0)
                        qe = pqe.tile([P, GS, C], F32, tag="qe")
                        for d_ in range(tb):
                            nc.tensor.matmul(qe[:, d_, :],
                                             lhsT=s12_t[:, t0 + d_, 1, :],
                                             rhs=q_sbuf[:, w, :], start=True, stop=False)
                            nc.tensor.matmul(qe[:, d_, :], lhsT=ident_f8[:],
                                             rhs=kv_g[:, t0 + d_, 0:C].bitcast(FP8),
                                             start=False, stop=True)
                        if gi == 1 and prev is not None:
                            if prev_attin is not None:
                                tail_b(*prev_attin)
                            prev_attin = (prev[0], tail_a(*prev))
                            prev = None
                        dsq = wk.tile([P, GS, C], BF16, tag="dsq")
                        nc.scalar.activation(dsq[:, 0:tb, :], qe[:, 0:tb, :], AF.Square)
                        # staged-halving reduce: two bf16 2x-mode adds, then a
                        # short TensorReduce (TensorReduce has no fast mode).
                        d4 = dsq[:, 0:tb, :].rearrange("p t (h j d) -> p t h j d", h=H, j=2)
                        h1 = wk.tile([P, GS, H, DH // 2], BF16, tag="h1")
                        nc.vector.tensor_add(h1[:, 0:tb, :, :], d4[:, :, :, 0, :], d4[:, :, :, 1, :])
                        h14 = h1[:, 0:tb, :, :].rearrange("p t h (j d) -> p t h j d", j=2)
                        h2 = wk.tile([P, GS, H, DH // 4], BF16, tag="h2")
                        nc.vector.tensor_add(h2[:, 0:tb, :, :], h14[:, :, :, 0, :], h14[:, :, :, 1, :])
                        s8 = wk.tile([P, GS, H], F32, tag="s8")
                        nc.vector.reduce_sum(
                            s8[:, 0:tb, :], h2[:, 0:tb, :, :], axis=AX.X)
                        sc = wk.tile([P, GS, H], F32, tag="sc")
                        nc.vector.scalar_tensor_tensor(
                            out=sc[:, 0:tb, :], in0=s8[:, 0:tb, :], scalar=-inv_s,
                            in1=rb_t[:, t0:t0 + tb, 4:12], op0=ALU.mult, op1=ALU.add)
                        eaux = wk.tile([P, GS, 48], BF16, tag="eaux")
                        nc.scalar.activation(
                            eaux[:, 0:tb, 0:16].rearrange("p t (h j) -> p t h j", h=H),
                            sc[:, 0:tb, :].unsqueeze(3).broadcast_to([P, tb, H, 2]),
                            AF.Exp)
                        pev = wk.tile([P, GS, C], BF16, tag="pev")
                        for d_ in range(tb):
                            nc.vector.tensor_mul(
                                pev[:, d_, :].rearrange("p (h a j) -> p h a j", h=H, j=2),
                                eaux[:, d_, 0:16].rearrange("p (h j) -> p h j", h=H)
                                    .unsqueeze(2).broadcast_to([P, H, DH // 2, 2]),
                                kv_g[:, t0 + d_, C:KVB].bitcast(BF16)
                                    .rearrange("p (h a j) -> p h a j", h=H, j=2))
                        nc.vector.tensor_mul(
                            eaux[:, 0:tb, 16:48].rearrange("p t (h a) -> p t h a", h=H),
                            eaux[:, 0:tb, 0:16].rearrange("p t (h j) -> p t h j", h=H)[:, :, :, 0:1]
                                .broadcast_to([P, tb, H, 4]),
                            rb_t[:, t0:t0 + tb, 0:4].unsqueeze(2)
                                .broadcast_to([P, tb, H, 4]))
                        for d_ in range(tb):
                            t = t0 + d_
                            nc.tensor.matmul(psV[:], lhsT=s12_t[:, t, 0, :],
                                             rhs=pev[:, d_, :],
                                             start=(t == 0), stop=False)
                            nc.tensor.matmul(psA[:], lhsT=s12_t[:, t, 0, :],
                                             rhs=eaux[:, d_, :],
                                             start=(t == 0), stop=(t == NT - 1))

                    prev = (w, psV, psA)
                if prev is not None:
                    if prev_attin is not None:
                        tail_b(*prev_attin)
                    prev_attin = (prev[0], tail_a(*prev))
                if prev_attin is not None:
                    tail_b(*prev_attin)

            # ============ P4: LN2 + FFN ============
            if "p4" not in _SKIP:
             with tc.tile_pool(name="p4", bufs=6) as wk, \
                 tc.tile_pool(name="p4tp", bufs=4, space="PSUM") as ptp, \
                 tc.tile_pool(name="f1ps", bufs=2, space="PSUM") as pps1, \
                 tc.tile_pool(name="f2ps", bufs=2, space="PSUM") as pps2:
                # mean/var from the Act-accumulated sums: mean = sx/C,
                # var = sx2/C - mean^2; one batched Sqrt (one table switch).
                mean_all = wk.tile([P, NW], F32, tag="mean_all")
                nc.vector.tensor_scalar_mul(mean_all[:], sx_all[:], 1.0 / C)
                msq = wk.tile([P, NW], F32, tag="msq")
                nc.vector.tensor_mul(msq[:], mean_all[:], mean_all[:])
                var_all = wk.tile([P, NW], F32, tag="var_all")
                nc.vector.scalar_tensor_tensor(
                    out=var_all[:], in0=sx2_all[:], scalar=1.0 / C,
                    in1=msq[:], op0=ALU.mult, op1=ALU.subtract)
                sd_all = wk.tile([P, NW], F32, tag="sd_all")
                nc.scalar.activation(sd_all[:], var_all[:], AF.Sqrt,
                                     bias=eps_t[:], scale=1.0)
                rs_all = wk.tile([P, NW], F32, tag="rs_all")
                nc.vector.reciprocal(rs_all[:], sd_all[:])
                nmr_all = wk.tile([P, NW], F32, tag="nmr_all")
                nc.vector.scalar_tensor_tensor(
                    out=nmr_all[:], in0=mean_all[:], scalar=-1.0,
                    in1=rs_all[:], op0=ALU.mult, op1=ALU.mult)
                def ffn1_chunk(rc):
                    for ht in range(HID // P):
                        ps = pps1.tile([P, 512], F32, tag="ps1")
                        for ko in range(0, C // P, 2):
                            nc.tensor.matmul(
                                ps[:], lhsT=w1_s[:, ko:ko + 2, ht * P:(ht + 1) * P],
                                rhs=zt_sbuf[:, ko:ko + 2, rc * 512:(rc + 1) * 512],
                                start=(ko == 0), stop=(ko == C // P - 2),
                                perf_mode=mybir.MatmulPerfMode.DoubleRow)
                        nc.scalar.activation(
                            hT_sbuf[:, ht, rc * 512:(rc + 1) * 512], ps[:],
                            AF.Gelu_apprx_tanh, bias=b1c_s[:, ht:ht + 1], scale=1.0 / WS)

                def ffn2_chunk(rc):
                    for m in range(rc * 4, rc * 4 + 4):
                        ps = pps2.tile([P, C], F32, tag="ps2")
                        for ht in range(0, HID // P, 2):
                            nc.tensor.matmul(ps[:], lhsT=hT_sbuf[:, ht:ht + 2, m * P:(m + 1) * P],
                                             rhs=w2_s[:, ht:ht + 2, :],
                                             start=(ht == 0), stop=False,
                                             perf_mode=mybir.MatmulPerfMode.DoubleRow)
                        nc.tensor.matmul(ps[:], lhsT=ones_s[:], rhs=brow_s[0:1, 2 * C:3 * C],
                                         start=False, stop=True)
                        if m % 2 == 0:
                            yt = wk.tile([P, 2, C], F32, tag="y", bufs=2)
                        nc.vector.scalar_tensor_tensor(
                            out=yt[:, m % 2, :], in0=ps[:], scalar=1.0 / WS,
                            in1=x2_all[:, m, :], op0=ALU.mult, op1=ALU.add)
                        if m % 2 == 1:
                            nc.sync.dma_start(y_t[:, m - 1:m + 1, :], yt[:])

                # LN2 windows with FFN chunks interleaved after every 4th.
                for m in range(NW):
                    zf = wk.tile([P, C], BF16, tag="ln_zf")
                    nc.vector.tensor_scalar(
                        out=zf[:], in0=x2_all[:, m, :],
                        scalar1=rs_all[:, m:m + 1], scalar2=nmr_all[:, m:m + 1],
                        op0=ALU.mult, op1=ALU.add)
                    tp = ptp.tile([P, C], BF16, tag="tp")
                    for c4 in range(C // P):
                        nc.tensor.transpose(tp[:, c4 * P:(c4 + 1) * P],
                                            zf[:, c4 * P:(c4 + 1) * P], ident[:])
                    if m % 2 == 0:
                        nc.scalar.activation(
                            zt_sbuf[:, :, m * P:(m + 1) * P],
                            tp[:].rearrange("p (c4 q) -> p c4 q", q=P), AF.Identity)
                    else:
                        nc.vector.tensor_copy(
                            zt_sbuf[:, :, m * P:(m + 1) * P],
                            tp[:].rearrange("p (c4 q) -> p c4 q", q=P))
                    if m % 4 == 3:
                        # FFN2 one chunk behind FFN1: its matmuls fill FFN1's
                        # gelu-wait bubbles.
                        ffn1_chunk(m // 4)
                        if m // 4 > 0:
                            ffn2_chunk(m // 4 - 1)
                ffn2_chunk(RS // 512 - 1)

    nc.compile()
    return nc


def _prep(inputs):
    row = np.asarray(inputs["row_index"]).astype(np.int64).ravel()
    col = np.asarray(inputs["col_index"]).astype(np.int64).ravel()
    tcol = np.asarray(inputs["to_col_index"]).astype(np.int64).ravel()
    bias = np.asarray(inputs["pos_att_bias"], dtype=np.float32)
    dist = np.asarray(inputs["dist"], dtype=np.float32).ravel()
    pos = np.asarray(inputs["pos"], dtype=np.float32)
    cpos = np.asarray(inputs["col_pos"], dtype=np.float32)

    # ---- balance rows into 128 bins (8 cores x 16 windows, 128 rows each) ---
    import heapq
    NB = NCORES * NW
    cnt = np.bincount(row, minlength=L)
    order_r = np.argsort(-cnt, kind="stable")
    heap = [(0, 0, b) for b in range(NB)]
    heapq.heapify(heap)
    bin_rows = [[] for _ in range(NB)]
    bin_sum = np.zeros(NB, np.int64)
    for r in order_r:
        popped = []
        while True:
            s, n, b = heapq.heappop(heap)
            if n < P:
                break
            popped.append((s, n, b))
        for x_ in popped:
            heapq.heappush(heap, x_)
        bin_rows[b].append(int(r))
        bin_sum[b] = s + cnt[r]
        heapq.heappush(heap, (int(bin_sum[b]), n + 1, b))

    # snake-assign bins to cores by edge-count rank; windows sorted descending
    # within each core so window j's count is similar across cores.
    rk = np.argsort(-bin_sum)
    core_bins = [[] for _ in range(NCORES)]
    for i, b in enumerate(rk):
        core_bins[i % NCORES].append(int(b))
    for c in range(NCORES):
        core_bins[c].sort(key=lambda b: -int(bin_sum[b]))
    counts = np.array([[bin_sum[b] for b in core_bins[c]] for c in range(NCORES)])
    nts = tuple(int(x) for x in np.ceil(counts.max(axis=0) / P).astype(int))

    # global row permutation: new row (c*RS + w*P + slot) = old row
    perm = np.empty(L, np.int64)
    for c in range(NCORES):
        for w in range(NW):
            b = core_bins[c][w]
            perm[c * RS + w * P:c * RS + (w + 1) * P] = bin_rows[b]
    inv_perm = np.empty(L, np.int64)
    inv_perm[perm] = np.arange(L)

    new_row = inv_perm[row]   # position of each edge's target row
    new_col = inv_perm[col]   # position of each edge's source col in permuted kvt

    TOT = sum(nts)
    E16 = sum(n * P // 16 for n in nts)
    toff = np.concatenate([[0], np.cumsum(nts)]).astype(int)
    eoff = np.concatenate([[0], np.cumsum([n * P // 16 for n in nts])]).astype(int)

    eidx_h = np.zeros((NCORES, P, E16), np.int16)
    rb_h = np.zeros((NCORES, P, TOT, 12), np.float32)
    rb_h[:, :, :, 4:12] = -1e4
    s12_h = np.zeros((NCORES, P, TOT, 2, P), np.float32)

    gw_all = new_row // P  # global window id (0..127) per edge
    order_e = np.argsort(gw_all, kind="stable")
    gw_s = gw_all[order_e]
    starts = np.searchsorted(gw_s, np.arange(NB + 1))
    for gw in range(NB):
        c, w = divmod(gw, NW)
        sl = order_e[starts[gw]:starts[gw + 1]]
        n = len(sl)
        if n == 0:
            continue
        TWw = nts[w] * P
        assert n <= TWw, (n, TWw)
        erows = (new_row[sl] - gw * P).astype(np.int64)
        ecols = new_col[sl]
        j = np.arange(n)
        wrap = np.zeros((16, TWw // 16), np.int16)
        wrap[j % 16, j // 16] = ecols.astype(np.int16)
        eidx_h[c, :, eoff[w]:eoff[w + 1]] = np.tile(wrap, (8, 1))
        t_of = toff[w] + j // P
        e_of = j % P
        rb_h[c, e_of, t_of, 0:3] = (cpos[tcol[sl]] - pos[row[sl]]) / dist[sl][:, None]
        rb_h[c, e_of, t_of, 3] = 1.0
        rb_h[c, e_of, t_of, 4:12] = bias[sl]
        s12_h[c, e_of, t_of, 0, erows] = 1.0   # s1: edge -> row scatter
        s12_h[c, erows, t_of, 1, e_of] = 1.0   # s2: row -> edge expand
    import ml_dtypes
    bf = ml_dtypes.bfloat16
    return nts, perm, eidx_h, rb_h.astype(bf), s12_h.astype(bf)


def kernel(**inputs):
    import ml_dtypes
    bf = ml_dtypes.bfloat16
    x = np.asarray(inputs["x"], dtype=np.float32)
    nts, perm, eidx_h, rb_h, s12_h = _prep(inputs)
    if nts not in _cache:
        _cache[nts] = _build(nts)
    nc = _cache[nts]

    f32 = lambda k: np.asarray(inputs[k], np.float32)
    g1, b1l = f32("ln1_g"), f32("ln1_b")
    g2, b2l = f32("ln2_g"), f32("ln2_b")
    Wq, Wk, Wv, Wo = f32("Wq"), f32("Wk"), f32("Wv"), f32("Wo")
    # Fold LN affine into the following matmuls; fold bk into bq (only the
    # difference q-k matters) and bv into bo (sum_e alpha = 1 per head).
    Wq_, Wk_, Wv_ = g1[:, None] * Wq, g1[:, None] * Wk, g1[:, None] * Wv
    bq_ = (b1l @ Wq + f32("bq")) - (b1l @ Wk + f32("bk"))
    bo_ = (b1l @ Wv + f32("bv")) @ Wo + f32("bo")
    W1_ = g2[:, None] * f32("W1")
    b1_ = b2l @ f32("W1") + f32("b1")
    import ml_dtypes as _md
    f8 = _md.float8_e4m3
    WS = 64.0
    # Wk negated: the kernel stores k pre-negated for the PE qe-k accumulate.
    w_qkv = (np.concatenate([Wq_, -Wk_, Wv_], axis=1) * WS).astype(f8)

    wv4 = np.concatenate([f32("Wvec"), f32("bvec")[None, :]], axis=0)
    w_vec4 = np.zeros((32, C), np.float32)
    for h in range(H):
        w_vec4[4 * h:4 * h + 4, h * DH:(h + 1) * DH] = wv4[:, h * DH:(h + 1) * DH]

    brows = np.zeros((1, 4 * C), np.float32)
    brows[0, 0:C] = bq_
    brows[0, C:2 * C] = bo_
    brows[0, 2 * C:3 * C] = f32("b2")
    b1_col = np.ascontiguousarray(b1_.reshape(HID // P, P).T)

    xp = x[perm]
    in_maps = []
    for c in range(NCORES):
        in_maps.append(dict(
            x_in=np.ascontiguousarray(xp[c * RS:(c + 1) * RS]),
            w_qkv=w_qkv, w_o=(Wo * WS).astype(f8),
            w_1=(W1_ * WS).astype(f8), w_2=(f32("W2") * WS).astype(f8),
            w_vec4=w_vec4.astype(bf), b1_col=b1_col,
            brows=(brows * WS).astype(f8),
            eidx=eidx_h[c], relbias=rb_h[c], s12=s12_h[c],
        ))
    _last["nc"] = nc
    _last["in_maps"] = in_maps
    res = run_bass_kernel_spmd(nc, in_maps, list(range(NCORES)))
    global _last_res
    _last_res = res
    yp = np.concatenate([res.results[c]["y_out"] for c in range(NCORES)], axis=0)
    y = np.empty_like(yp)
    y[perm] = yp
    return np.asarray(y, np.float32)


_last = {}
_last_res = None
